# revision 33
# baseline (speedup 1.0000x reference)
"""PhysicsInformedLoss on 8 Trainium2 NeuronCores.

Sharding strategy (degree-class padded CSR):
- Edges are grouped by destination node `row` (the scatter target of every
  segment-mean in the reference). Nodes with deg>0 are binned into degree
  classes K (multiples of 4, small classes merged); each node gets exactly K
  contiguous "slots" (its edges + self-pads, pads contribute exactly 0).
- Nodes of each class are split evenly across the 8 cores (identical padded
  per-core counts -> one SPMD program). Per core, node i of a class maps to
  partition p = i // q (q nodes/partition), so every per-node segment sum is
  a static strided reduction along the free dimension.
- The host gathers the 7 col-side planes (pos xyz, vel uvw, p) in slot order
  (this is the "shard the edges" data layout step); the row side is the
  per-node resident plane broadcast along K by a stride-0 access pattern.
- Device per core: stream col planes, compute all per-edge terms, strided
  per-node reduce, finish div/residual/squares; output per-partition partial
  sums. Host sums 8x128 partials and forms the scalar loss.
"""
import contextlib
import ctypes
import os
import sys
import tempfile
import types

import numpy as np

import concourse.bass as bass
import concourse.tile as tile
from concourse import mybir
from concourse.vector_clock import ScopedClock
from concourse.bass_utils import run_bass_kernel_spmd

N_CORES = 8
P = 128
EPS = 1e-8
REYNOLDS = 1000000.0
LAMBDA_CONT = 0.1
LAMBDA_MOM = 0.01
F_TILE = 1024  # target per-partition columns per tile

# ---------------------------------------------------------------- tile patch
# walrus in this environment allows only ONE sync-wait per instruction, but
# Tile's scheduler can emit several. Split surplus waits onto engine NOPs
# inserted right before the offending instruction.
_MAX_WAITS = 1


def _split_multi_waits(nc, handles):
    work = []
    for fn in nc.m.functions:
        for bb in fn.blocks:
            items = []
            for inst in bb.instructions:
                si = inst.sync_info
                waits = list(si.on_wait) if si and si.on_wait else []
                if len(waits) > _MAX_WAITS:
                    keep = len(waits) - _MAX_WAITS
                    extra = waits[:keep]
                    si.on_wait = waits[keep:]
                    chunks = [
                        extra[i : i + _MAX_WAITS]
                        for i in range(0, len(extra), _MAX_WAITS)
                    ]
                    items.append((inst.name, inst.engine, chunks))
            if items:
                work.append((bb, items))
    if not work:
        return
    created = {}
    placements = {}
    for bb, items in work:
        plc = {}
        for inst_name, engine, chunks in items:
            nops = []
            for chunk in chunks:
                for w in chunk:
                    h = handles.get(w.ant_name)
                    assert h is not None, f"no sem handle for {w.ant_name}"
                    ni = nc.engines[engine].wait_ge(h, w.wait_value)
                    created[ni.ins.name] = None
                    nops.append(ni.ins)
            plc[inst_name] = nops
        placements[id(bb)] = plc
    for fn in nc.m.functions:
        for bb in fn.blocks:
            plc = placements.get(id(bb), {})
            newlist = []
            for inst in bb.instructions:
                if inst.name in created:
                    continue
                if inst.name in plc:
                    newlist.extend(plc[inst.name])
                newlist.append(inst)
            bb.instructions = newlist


def _patched_drain_and_barrier(self, tick_clock, wait_clock):
    drain_inst = self.nc.sync.drain()
    wait_clock.add_sem_waits(
        drain_inst.ins, ScopedClock({None: tick_clock.global_clock})
    )
    handles = {h.name: h for h in self.sems.allocated().values()}
    _split_multi_waits(self.nc, handles)
    self.nc.all_engine_barrier()
    popped = self.nc._tile_sem_poison_stack.pop()
    assert popped is self._sem_poison
    self.nc.clear_and_free_semaphores(list(self.sems.allocated().values()))
    self.nc.all_engine_barrier()


tile.TileContext._drain_and_barrier = _patched_drain_and_barrier

# ------------------------------------------------------------- ntff hook
# The env's antenv package lacks axon_hooks; recreate the NTFF profile hook
# via ctypes so run_bass_kernel_spmd(trace=True) works (test/profiling only).
_AXON_SO = "/opt/axon/libaxon_pjrt.so"


def _install_ntff_hook():
    if "antenv.axon_hooks" in sys.modules:
        return
    try:
        lib = ctypes.CDLL(_AXON_SO)
        lib.axon_start_nrt_profile.argtypes = [
            ctypes.POINTER(ctypes.c_int64),
            ctypes.c_size_t,
        ]
        lib.axon_start_nrt_profile.restype = ctypes.c_int64
        lib.axon_stop_nrt_profile.argtypes = [ctypes.c_char_p]
        lib.axon_stop_nrt_profile.restype = ctypes.c_int64
    except Exception:
        return

    @contextlib.contextmanager
    def _hook(output_dir, device_ids):
        import jax

        jax.devices()
        if device_ids:
            ids = (ctypes.c_int64 * len(device_ids))(*device_ids)
            rc = lib.axon_start_nrt_profile(ids, len(device_ids))
        else:
            rc = lib.axon_start_nrt_profile(None, 0)
        if rc != 0:
            raise RuntimeError(f"axon_start_nrt_profile rc={rc}")
        try:
            yield
        finally:
            n = lib.axon_stop_nrt_profile(str(output_dir).encode())
            print(f"profile: {n} file(s) written to {output_dir}", file=sys.stderr)

    mod = types.ModuleType("antenv.axon_hooks")
    mod.get_axon_ntff_profile_hook = lambda: _hook
    mod.set_axon_ntff_profile_hook = lambda h: None
    sys.modules["antenv.axon_hooks"] = mod


# ---------------------------------------------------------------- host prep


def _build_plan(row, col, pos, n, tau=1e-2, knear=8, min_class_slots=128 * 1024):
    """Split edges into far (bf16-safe, dist^2 >= tau) and near (exact fp32
    section, <= knear per node; overflow demoted to far). Returns plan with
    per-class far geometry + near geometry."""
    deg = np.bincount(row, minlength=n).astype(np.int64)
    order = np.argsort(row, kind="stable")
    row_s = row[order]
    col_s = col[order]
    pd = pos[col_s].astype(np.float32) - pos[row_s].astype(np.float32)
    r2e = (pd * pd).sum(1)
    near_e = r2e < np.float32(tau)
    # within each node's run, put near edges first (stable)
    order2 = np.argsort(row_s * 2 + (~near_e), kind="stable")
    col_s2 = col_s[order2]
    offs = np.zeros(n + 1, dtype=np.int64)
    np.cumsum(deg, out=offs[1:])
    dnear = np.bincount(row_s[near_e], minlength=n)
    dnear = np.minimum(dnear, knear).astype(np.int64)
    dfar = deg - dnear
    affected = dnear > 0

    active = deg > 0
    kraw = ((np.maximum(dfar, 1) + 3) // 4) * 4
    uniq = np.unique(kraw[active])
    groups = []
    pend = []
    pend_slots = 0
    for K in uniq:
        ids = np.nonzero(active & (kraw == K))[0]
        pend.append(ids)
        pend_slots += ids.size * int(K)
        if pend_slots >= min_class_slots or K == uniq[-1]:
            allids = np.concatenate(pend)
            groups.append((int(K), allids))
            pend = []
            pend_slots = 0

    plan = []
    for K, ids in groups:
        aff = affected[ids]
        # affected first (stable), then round-robin core split
        ids = ids[np.argsort(~aff, kind="stable")]
        per_core = -(-ids.size // N_CORES)
        m = -(-per_core // P) * P  # per-core nodes, padded to mult of 128
        a_max = 0
        for c in range(N_CORES):
            a_max = max(a_max, int(affected[ids[c::N_CORES]].sum()))
        w = -(-a_max // P)  # near cols per class (common)
        plan.append((K, ids, m, w))
    return plan, deg, dnear, dfar, offs, col_s2, knear


def _build_streams(plan, deg, dnear, dfar, offs, col_s2, knear, nodedata):
    """nodedata: [n,7] f32. Builds per-core far bf16 streams (interleaved
    node->partition mapping) + near fp32 section. Returns cores, NN, S, Wq."""
    import ml_dtypes

    bf = ml_dtypes.bfloat16
    S = sum(m * K for K, _, m, _ in plan)
    NN = sum(m for _, _, m, _ in plan)
    Wq = sum(w for _, _, _, w in plan)
    cores = []
    for c in range(N_CORES):
        colp = np.zeros((7, P, S // P), bf)
        nodb = np.zeros((7, P, NN // P), bf)
        cnt = np.ones((P, NN // P), np.float32)
        nearA = np.zeros((7, P, Wq * knear), np.float32)
        nearB = np.zeros((7, P, Wq * knear), np.float32)
        off_slots = 0
        off_nodes = 0
        off_q = 0
        for K, ids, m, w in plan:
            q = m // P
            ids_c = ids[c::N_CORES]
            k_real = ids_c.size
            # ---- far stream ----
            vals = np.zeros((m, K, 7), np.float32)
            nodevals = np.zeros((m, 7), np.float32)
            cv = np.ones(m, np.float32)
            if k_real > 0:
                colmat = np.empty((k_real, K), np.int64)
                colmat[:] = ids_c[:, None]
                df = dfar[ids_c]
                oo = offs[ids_c] + dnear[ids_c]  # far edges after near ones
                ar = np.arange(K)[None, :]
                valid = ar < df[:, None]
                src_idx = (oo[:, None] + ar)[valid]
                colmat[valid] = col_s2[src_idx]
                vals[:k_real] = nodedata[colmat]
                nodevals[:k_real] = nodedata[ids_c]
                cv[:k_real] = np.maximum(deg[ids_c], 1).astype(np.float32)
            wv = vals.reshape(q, P, K, 7).transpose(3, 1, 0, 2).reshape(7, P, q * K)
            colp[:, :, off_slots : off_slots + q * K] = wv.astype(bf)
            nodb[:, :, off_nodes : off_nodes + q] = (
                nodevals.reshape(q, P, 7).transpose(2, 1, 0).astype(bf)
            )
            cnt[:, off_nodes : off_nodes + q] = cv.reshape(q, P).T
            # ---- near section (affected nodes = first a of ids_c) ----
            if w > 0:
                mw = w * P
                a = int((dnear[ids_c] > 0).sum()) if k_real > 0 else 0
                nvals = np.zeros((mw, knear, 7), np.float32)
                nnode = np.zeros((mw, knear, 7), np.float32)
                if a > 0:
                    aid = ids_c[:a]
                    colm = np.empty((a, knear), np.int64)
                    colm[:] = aid[:, None]
                    dn = dnear[aid]
                    oo = offs[aid]
                    ar = np.arange(knear)[None, :]
                    valid = ar < dn[:, None]
                    src_idx = (oo[:, None] + ar)[valid]
                    colm[valid] = col_s2[src_idx]
                    nvals[:a] = nodedata[colm]
                    nnode[:a] = nodedata[aid][:, None, :]
                wa = nvals.reshape(w, P, knear, 7).transpose(3, 1, 0, 2)
                wb = nnode.reshape(w, P, knear, 7).transpose(3, 1, 0, 2)
                nearA[:, :, off_q * knear : (off_q + w) * knear] = wa.reshape(
                    7, P, w * knear
                )
                nearB[:, :, off_q * knear : (off_q + w) * knear] = wb.reshape(
                    7, P, w * knear
                )
            off_slots += q * K
            off_nodes += q
            off_q += w
        cores.append(dict(colp=colp, nodb=nodb, cnt=cnt, nearA=nearA, nearB=nearB))
    return cores, NN, S, Wq


# ---------------------------------------------------------------- bass build


def _class_tiles(plan):
    """Yield (K, g_nodes_in_tile, slot_col_offset, node_col_offset) splits."""
    tiles = []
    off_s = 0
    off_n = 0
    for K, _, m, _ in plan:
        q = m // P
        gmax = max(1, F_TILE // K)
        i = 0
        while i < q:
            g = min(gmax, q - i)
            tiles.append((K, g, off_s + i * K, off_n + i))
            i += g
        off_s += q * K
        off_n += q
    return tiles


def _class_merges(plan):
    """Per-class (node_col_offset, near_col_offset, w) for near merges."""
    merges = []
    off_n = 0
    off_q = 0
    for K, _, m, w in plan:
        if w > 0:
            merges.append((off_n, off_q, w))
        off_n += m // P
        off_q += w
    return merges


def _raw_scalar_act(nc, out, in_, func, bias=0.0, scale=1.0):
    """InstActivation without the python wrapper's Reciprocal ban and without
    the const-AP bias conversion (immediates work for these funcs here)."""
    inputs = [nc.scalar.lower_ap(in_)]
    for arg in (bias, scale, 0.0):
        inputs.append(mybir.ImmediateValue(dtype=mybir.dt.float32, value=arg))
    return nc.scalar.add_instruction(
        mybir.InstActivation(
            name=nc.get_next_instruction_name(),
            func=func,
            ins=inputs,
            outs=[nc.scalar.lower_ap(out)],
        )
    )


def _build_nc(plan, NN, S, Wq, KN, DQ):
    """SPMD bass program: bf16 far pipeline + small exact fp32 near section."""
    fp32 = mybir.dt.float32
    bf16 = mybir.dt.bfloat16
    nc = bass.Bass("TRN2", target_bir_lowering=False)
    W = S // P
    Q = NN // P
    WN = max(Wq, 1) * KN

    cols = [
        nc.dram_tensor(f"col{i}", [P, W], bf16, kind="ExternalInput")
        for i in range(7)
    ]
    nodb = nc.dram_tensor("nodb", [P, 7 * Q], bf16, kind="ExternalInput")
    cntT = nc.dram_tensor("cnt", [P, Q], fp32, kind="ExternalInput")
    nearA = nc.dram_tensor("nearA", [P, 7 * WN], fp32, kind="ExternalInput")
    nearB = nc.dram_tensor("nearB", [P, 7 * WN], fp32, kind="ExternalInput")
    dlp = nc.dram_tensor("dlp", [P, 4 * DQ], fp32, kind="ExternalInput")
    dlt = nc.dram_tensor("dlt", [P, 4 * DQ], fp32, kind="ExternalInput")
    out = nc.dram_tensor("out", [P, 8], fp32, kind="ExternalOutput")

    AF = mybir.ActivationFunctionType
    OP = mybir.AluOpType

    with tile.TileContext(nc) as tc:
        with (
            tc.tile_pool(name="resident", bufs=1) as res_pool,
            tc.tile_pool(name="colp", bufs=2) as col_pool,
            tc.tile_pool(name="exp", bufs=2) as exp_pool,
            tc.tile_pool(name="tmp", bufs=2) as tmp_pool,
            tc.tile_pool(name="tmp1", bufs=2) as tmp1_pool,
        ):
            nodbt = res_pool.tile([P, 7 * Q], bf16)
            nc.sync.dma_start(nodbt[:], nodb.ap()[:])
            cntt = res_pool.tile([P, Q], fp32)
            nc.sync.dma_start(cntt[:], cntT.ap()[:])
            acc = res_pool.tile([P, 7 * Q], fp32)  # vg, lu, lv, lw, px, py, pz

            # ---- data loss (small, fp32) ----
            dlpt = res_pool.tile([P, 4 * DQ], fp32)
            nc.sync.dma_start(dlpt[:], dlp.ap()[:])
            dltt = res_pool.tile([P, 4 * DQ], fp32)
            nc.sync.dma_start(dltt[:], dlt.ap()[:])
            dld = res_pool.tile([P, 4 * DQ], fp32)
            nc.vector.tensor_sub(dld[:], dlpt[:], dltt[:])
            acc_vel = res_pool.tile([P, 1], fp32)
            acc_pres = res_pool.tile([P, 1], fp32)
            nc.scalar.activation(
                dld[:, 0 : 3 * DQ], dld[:, 0 : 3 * DQ], AF.Square,
                accum_out=acc_vel[:],
            )
            nc.scalar.activation(
                dld[:, 3 * DQ : 4 * DQ], dld[:, 3 * DQ : 4 * DQ], AF.Square,
                accum_out=acc_pres[:],
            )

            # ---- far loop (all bf16) ----
            acc4 = acc[:].rearrange("p (i q) -> p i q", i=7, q=Q)
            for K, g, off_s, off_n in _class_tiles(plan):
                F = g * K
                cvt = col_pool.tile([P, 7 * F], bf16, tag="cvt", name="cvt")
                for i in range(7):
                    nc.sync.dma_start(
                        cvt[:, i * F : (i + 1) * F],
                        cols[i].ap()[:, off_s : off_s + F],
                    )

                # node expand, all 7 planes in one ScalarE op
                evt = exp_pool.tile([P, 7 * F], bf16, tag="evt", name="evt")
                a7 = nodbt[:].rearrange("p (i q) -> p i q", i=7, q=Q)
                nc.scalar.copy(
                    evt[:].rearrange("p (i g k) -> p i g k", i=7, g=g, k=K),
                    a7[:, :, off_n : off_n + g]
                    .unsqueeze(-1)
                    .broadcast_to([P, 7, g, K]),
                )

                # all 7 diffs in one bf16 sub (2x)
                dft = tmp1_pool.tile([P, 7 * F], bf16, tag="dft", name="dft")
                nc.vector.tensor_sub(dft[:], cvt[:], evt[:])

                # r2 = dx^2+dy^2+dz^2 in bf16
                sqt = tmp_pool.tile([P, 3 * F], bf16, tag="sqt", name="sqt")
                nc.vector.tensor_mul(sqt[:], dft[:, 0 : 3 * F], dft[:, 0 : 3 * F])
                nc.vector.tensor_add(sqt[:, 0:F], sqt[:, 0:F], sqt[:, F : 2 * F])
                nc.vector.tensor_add(
                    sqt[:, 0:F], sqt[:, 0:F], sqt[:, 2 * F : 3 * F]
                )
                r2 = sqt[:, 0:F]

                # d1 = 1/(sqrt(r2)+eps), d2 = 1/(r2+eps) via ScalarE LUTs
                ss = tmp_pool.tile([P, F], bf16, tag="ss", name="ss")
                nc.scalar.activation(ss[:], r2, AF.Sqrt)
                d1 = tmp_pool.tile([P, F], bf16, tag="d1")
                _raw_scalar_act(nc, d1[:], ss[:], AF.Reciprocal, bias=EPS)
                d2 = tmp_pool.tile([P, F], bf16, tag="d2")
                _raw_scalar_act(nc, d2[:], r2, AF.Reciprocal, bias=EPS)

                vals = tmp1_pool.tile([P, 7 * F], bf16, tag="vals", name="vals")
                # g = sum duvw*dxyz ; velgrad = g*d1 -> vals plane 0
                prt = tmp1_pool.tile([P, 3 * F], bf16, tag="prt", name="prt")
                nc.vector.tensor_mul(
                    prt[:], dft[:, 3 * F : 6 * F], dft[:, 0 : 3 * F]
                )
                nc.vector.tensor_add(prt[:, 0:F], prt[:, 0:F], prt[:, F : 2 * F])
                nc.vector.tensor_add(
                    prt[:, 0:F], prt[:, 0:F], prt[:, 2 * F : 3 * F]
                )
                nc.vector.tensor_mul(vals[:, 0:F], prt[:, 0:F], d1[:])

                # cp = dq*d1*d1 (in-place in dft plane 6)
                cpq = dft[:, 6 * F : 7 * F]
                nc.vector.tensor_mul(cpq, cpq, d1[:])
                nc.vector.tensor_mul(cpq, cpq, d1[:])

                # pg = cp * dxyz -> vals planes 4:7 (cp broadcast over 3)
                cp3 = bass.AP(dft[:].tensor, dft[:].offset + 6 * F,
                              [dft[:].ap[0], [0, 3], [1, F]])
                nc.vector.tensor_mul(vals[:, 4 * F : 7 * F].rearrange(
                    "p (i f) -> p i f", i=3, f=F), cp3,
                    dft[:, 0 : 3 * F].rearrange("p (i f) -> p i f", i=3, f=F))
                # lap = duvw * d2 -> vals planes 1:4 (d2 broadcast over 3)
                d23 = bass.AP(d2[:].tensor, d2[:].offset,
                              [d2[:].ap[0], [0, 3], [1, F]])
                nc.vector.tensor_mul(vals[:, 1 * F : 4 * F].rearrange(
                    "p (i f) -> p i f", i=3, f=F),
                    dft[:, 3 * F : 6 * F].rearrange("p (i f) -> p i f", i=3, f=F),
                    d23)

                # segment sums: aligned halving tree over K down to 2
                v4 = vals[:].rearrange("p (i g k) -> p i g k", i=7, g=g, k=K)
                k = K
                while k > 2:
                    if k % 4 == 2:
                        nc.vector.tensor_add(
                            v4[:, :, :, 0:2], v4[:, :, :, 0:2],
                            v4[:, :, :, k - 2 : k],
                        )
                        k -= 2
                    else:
                        h = k // 2
                        nc.vector.tensor_add(
                            v4[:, :, :, 0:h], v4[:, :, :, 0:h],
                            v4[:, :, :, h:k],
                        )
                        k = h
                nc.vector.tensor_reduce(
                    acc4[:, :, off_n : off_n + g],
                    v4[:, :, :, 0:k],
                    mybir.AxisListType.X,
                    OP.add,
                )

            # ---- near section (exact fp32, tiny) ----
            if Wq > 0:
                nat = col_pool.tile([P, 7 * WN], fp32, tag="cvt", name="nat")
                nc.sync.dma_start(nat[:], nearA.ap()[:])
                nbt = exp_pool.tile([P, 7 * WN], fp32, tag="evt", name="nbt")
                nc.sync.dma_start(nbt[:], nearB.ap()[:])
                nd = tmp1_pool.tile([P, 7 * WN], fp32, tag="dft", name="nd")
                nc.vector.tensor_sub(nd[:], nat[:], nbt[:])
                nsq = tmp_pool.tile([P, 3 * WN], fp32, tag="sqt", name="nsq")
                nc.scalar.activation(nsq[:], nd[:, 0 : 3 * WN], AF.Square)
                nc.vector.tensor_add(
                    nsq[:, 0:WN], nsq[:, 0:WN], nsq[:, WN : 2 * WN]
                )
                nc.vector.tensor_add(
                    nsq[:, 0:WN], nsq[:, 0:WN], nsq[:, 2 * WN : 3 * WN]
                )
                r2n = nsq[:, 0:WN]
                ssn = tmp_pool.tile([P, WN], fp32, tag="ss", name="ssn")
                nc.scalar.activation(ssn[:], r2n, AF.Sqrt)
                d1n = tmp_pool.tile([P, WN], fp32, tag="d1", name="d1n")
                _raw_scalar_act(nc, d1n[:], ssn[:], AF.Reciprocal, bias=EPS)
                d2n = tmp_pool.tile([P, WN], fp32, tag="d2", name="d2n")
                _raw_scalar_act(nc, d2n[:], r2n, AF.Reciprocal, bias=EPS)

                nvals = tmp1_pool.tile([P, 7 * WN], fp32, tag="vals", name="nvals")
                prn = tmp1_pool.tile([P, 3 * WN], fp32, tag="prt", name="prn")
                nc.vector.tensor_mul(
                    prn[:], nd[:, 3 * WN : 6 * WN], nd[:, 0 : 3 * WN]
                )
                nc.vector.tensor_add(
                    prn[:, 0:WN], prn[:, 0:WN], prn[:, WN : 2 * WN]
                )
                nc.vector.tensor_add(
                    prn[:, 0:WN], prn[:, 0:WN], prn[:, 2 * WN : 3 * WN]
                )
                nc.vector.tensor_mul(nvals[:, 0:WN], prn[:, 0:WN], d1n[:])
                cpn = nd[:, 6 * WN : 7 * WN]
                nc.vector.tensor_mul(cpn, cpn, d1n[:])
                nc.vector.tensor_mul(cpn, cpn, d1n[:])
                cp3n = bass.AP(nd[:].tensor, nd[:].offset + 6 * WN,
                               [nd[:].ap[0], [0, 3], [1, WN]])
                nc.vector.tensor_mul(
                    nvals[:, 4 * WN : 7 * WN].rearrange(
                        "p (i f) -> p i f", i=3, f=WN),
                    cp3n,
                    nd[:, 0 : 3 * WN].rearrange("p (i f) -> p i f", i=3, f=WN))
                d23n = bass.AP(d2n[:].tensor, d2n[:].offset,
                               [d2n[:].ap[0], [0, 3], [1, WN]])
                nc.vector.tensor_mul(
                    nvals[:, 1 * WN : 4 * WN].rearrange(
                        "p (i f) -> p i f", i=3, f=WN),
                    nd[:, 3 * WN : 6 * WN].rearrange("p (i f) -> p i f", i=3, f=WN),
                    d23n)

                accN = res_pool.tile([P, 7 * Wq], fp32)
                nv4 = nvals[:].rearrange(
                    "p (i w k) -> p i w k", i=7, w=Wq, k=KN
                )
                k = KN
                while k > 2:
                    h = k // 2
                    nc.vector.tensor_add(
                        nv4[:, :, :, 0:h], nv4[:, :, :, 0:h], nv4[:, :, :, h:k]
                    )
                    k = h
                accN4 = accN[:].rearrange("p (i w) -> p i w", i=7, w=Wq)
                nc.vector.tensor_reduce(
                    accN4, nv4[:, :, :, 0:k], mybir.AxisListType.X, OP.add
                )
                # merge near partial sums into acc (aligned contiguous ranges)
                for off_n, off_q, w in _class_merges(plan):
                    nc.vector.tensor_add(
                        acc4[:, :, off_n : off_n + w],
                        acc4[:, :, off_n : off_n + w],
                        accN4[:, :, off_q : off_q + w],
                    )

            # ---- finish (fp32, small) ----
            icnt = res_pool.tile([P, Q], fp32)
            _raw_scalar_act(nc, icnt[:], cntt[:], AF.Reciprocal)
            div = res_pool.tile([P, Q], fp32)
            nc.vector.tensor_mul(div[:], acc[:, 0:Q], icnt[:])
            acc_div2 = res_pool.tile([P, 1], fp32)
            nc.scalar.activation(div[:], div[:], AF.Square, accum_out=acc_div2[:])
            acc_m = [
                res_pool.tile([P, 1], fp32, tag=f"am{i}", name=f"am{i}")
                for i in range(3)
            ]
            for i in range(3):
                r = res_pool.tile([P, Q], fp32, tag="rfin")
                nc.vector.scalar_tensor_tensor(
                    r[:],
                    acc[:, (1 + i) * Q : (2 + i) * Q],
                    1.0 / REYNOLDS,
                    acc[:, (4 + i) * Q : (5 + i) * Q],
                    OP.mult,
                    OP.add,
                )
                nc.vector.tensor_mul(r[:], r[:], icnt[:])
                nc.scalar.activation(r[:], r[:], AF.Square, accum_out=acc_m[i][:])

            outt = res_pool.tile([P, 8], fp32)
            nc.vector.memset(outt[:], 0.0)
            nc.vector.tensor_copy(outt[:, 0:1], acc_vel[:])
            nc.vector.tensor_copy(outt[:, 1:2], acc_pres[:])
            nc.vector.tensor_copy(outt[:, 2:3], acc_div2[:])
            nc.vector.tensor_copy(outt[:, 3:4], acc_m[0][:])
            nc.vector.tensor_copy(outt[:, 4:5], acc_m[1][:])
            nc.vector.tensor_copy(outt[:, 5:6], acc_m[2][:])
            nc.sync.dma_start(out.ap()[:], outt[:])

    return nc


# ---------------------------------------------------------------- entry

_CACHE = {}


def _get_nc(key, plan, NN, S, Wq, KN, DQ):
    if key not in _CACHE:
        _CACHE[key] = _build_nc(plan, NN, S, Wq, KN, DQ)
    return _CACHE[key]


LAST_RESULT = None  # BassKernelResults of the most recent run (for profiling)


def kernel(pred, target, edge_index, pos, _trace_dir=None):
    global LAST_RESULT
    pred = np.asarray(pred)
    target = np.asarray(target)
    pos = np.asarray(pos).astype(np.float32)
    row = np.asarray(edge_index[0]).astype(np.int64)
    col = np.asarray(edge_index[1]).astype(np.int64)
    n = pred.shape[0]

    plan, deg, dnear, dfar, offs, col_s2, KN = _build_plan(row, col, pos, n)
    nodedata = np.concatenate(
        [pos, pred.astype(np.float32)], axis=1
    )
    cores, NN, S, Wq = _build_streams(
        plan, deg, dnear, dfar, offs, col_s2, KN, nodedata
    )
    Qn = NN // P
    WN = max(Wq, 1) * KN

    # data-loss slices: split all n nodes across cores, pad to mult of 128
    per = -(-n // N_CORES)
    DQ = -(-per // P)
    predf = pred.astype(np.float32)
    targf = target.astype(np.float32)

    in_maps = []
    for c in range(N_CORES):
        lo, hi = c * per, min((c + 1) * per, n)
        ps = np.zeros((P * DQ, 4), np.float32)
        ts = np.zeros((P * DQ, 4), np.float32)
        ps[: hi - lo] = predf[lo:hi]
        ts[: hi - lo] = targf[lo:hi]
        # [P, 4*DQ] with plane-major columns: plane i at cols [i*DQ, (i+1)*DQ)
        dlp = ps.reshape(P, DQ, 4).transpose(0, 2, 1).reshape(P, 4 * DQ)
        dlt = ts.reshape(P, DQ, 4).transpose(0, 2, 1).reshape(P, 4 * DQ)
        cc = cores[c]
        m = dict(
            cnt=np.ascontiguousarray(cc["cnt"]),
            nodb=np.ascontiguousarray(
                cc["nodb"].transpose(1, 0, 2).reshape(P, 7 * Qn)
            ),
            nearA=np.ascontiguousarray(
                cc["nearA"].transpose(1, 0, 2).reshape(P, 7 * (Wq * KN))
            ) if Wq > 0 else np.zeros((P, 7 * WN), np.float32),
            nearB=np.ascontiguousarray(
                cc["nearB"].transpose(1, 0, 2).reshape(P, 7 * (Wq * KN))
            ) if Wq > 0 else np.zeros((P, 7 * WN), np.float32),
            dlp=np.ascontiguousarray(dlp),
            dlt=np.ascontiguousarray(dlt),
        )
        for i in range(7):
            m[f"col{i}"] = np.ascontiguousarray(cc["colp"][i])
        in_maps.append(m)

    key = (tuple((K, mm, w) for K, _, mm, w in plan), NN, S, Wq, KN, DQ)
    plan_geom = [(K, None, mm, w) for K, _, mm, w in plan]
    nc = _get_nc(key, plan_geom, NN, S, Wq, KN, DQ)

    if _trace_dir is not None:
        _install_ntff_hook()
        res = run_bass_kernel_spmd(
            nc, in_maps, core_ids=list(range(N_CORES)), trace=True,
            tmpdir=_trace_dir,
        )
    else:
        res = run_bass_kernel_spmd(nc, in_maps, core_ids=list(range(N_CORES)))
    LAST_RESULT = res

    tot = np.zeros(8, np.float64)
    for c in range(N_CORES):
        tot += res.results[c]["out"].astype(np.float64).sum(axis=0)
    s_vel, s_pres, s_div2, am0, am1, am2 = (
        tot[0], tot[1], tot[2], tot[3], tot[4], tot[5]
    )
    loss = (
        s_vel / (3 * n)
        + s_pres / n
        + LAMBDA_CONT * s_div2 / n
        + LAMBDA_MOM * (am0 + am1 + am2) / (3 * n)
    )
    return np.float32(loss)


# revision 34
# speedup vs baseline: 1.0905x; 1.0905x over previous
"""PhysicsInformedLoss on 8 Trainium2 NeuronCores.

Sharding strategy (degree-class padded CSR):
- Edges are grouped by destination node `row` (the scatter target of every
  segment-mean in the reference). Nodes with deg>0 are binned into degree
  classes K (multiples of 4, small classes merged); each node gets exactly K
  contiguous "slots" (its edges + self-pads, pads contribute exactly 0).
- Nodes of each class are split evenly across the 8 cores (identical padded
  per-core counts -> one SPMD program). Per core, node i of a class maps to
  partition p = i // q (q nodes/partition), so every per-node segment sum is
  a static strided reduction along the free dimension.
- The host gathers the 7 col-side planes (pos xyz, vel uvw, p) in slot order
  (this is the "shard the edges" data layout step); the row side is the
  per-node resident plane broadcast along K by a stride-0 access pattern.
- Device per core: stream col planes, compute all per-edge terms, strided
  per-node reduce, finish div/residual/squares; output per-partition partial
  sums. Host sums 8x128 partials and forms the scalar loss.
"""
import contextlib
import ctypes
import os
import sys
import tempfile
import types

import numpy as np

import concourse.bass as bass
import concourse.tile as tile
from concourse import mybir
from concourse.vector_clock import ScopedClock
from concourse.bass_utils import run_bass_kernel_spmd

N_CORES = 8
P = 128
EPS = 1e-8
REYNOLDS = 1000000.0
LAMBDA_CONT = 0.1
LAMBDA_MOM = 0.01
F_TILE = 1408  # target per-partition columns per tile

# ---------------------------------------------------------------- tile patch
# walrus in this environment allows only ONE sync-wait per instruction, but
# Tile's scheduler can emit several. Split surplus waits onto engine NOPs
# inserted right before the offending instruction.
_MAX_WAITS = 1


def _split_multi_waits(nc, handles):
    work = []
    for fn in nc.m.functions:
        for bb in fn.blocks:
            items = []
            for inst in bb.instructions:
                si = inst.sync_info
                waits = list(si.on_wait) if si and si.on_wait else []
                if len(waits) > _MAX_WAITS:
                    keep = len(waits) - _MAX_WAITS
                    extra = waits[:keep]
                    si.on_wait = waits[keep:]
                    chunks = [
                        extra[i : i + _MAX_WAITS]
                        for i in range(0, len(extra), _MAX_WAITS)
                    ]
                    items.append((inst.name, inst.engine, chunks))
            if items:
                work.append((bb, items))
    if not work:
        return
    created = {}
    placements = {}
    for bb, items in work:
        plc = {}
        for inst_name, engine, chunks in items:
            nops = []
            for chunk in chunks:
                for w in chunk:
                    h = handles.get(w.ant_name)
                    assert h is not None, f"no sem handle for {w.ant_name}"
                    ni = nc.engines[engine].wait_ge(h, w.wait_value)
                    created[ni.ins.name] = None
                    nops.append(ni.ins)
            plc[inst_name] = nops
        placements[id(bb)] = plc
    for fn in nc.m.functions:
        for bb in fn.blocks:
            plc = placements.get(id(bb), {})
            newlist = []
            for inst in bb.instructions:
                if inst.name in created:
                    continue
                if inst.name in plc:
                    newlist.extend(plc[inst.name])
                newlist.append(inst)
            bb.instructions = newlist


def _patched_drain_and_barrier(self, tick_clock, wait_clock):
    drain_inst = self.nc.sync.drain()
    wait_clock.add_sem_waits(
        drain_inst.ins, ScopedClock({None: tick_clock.global_clock})
    )
    handles = {h.name: h for h in self.sems.allocated().values()}
    _split_multi_waits(self.nc, handles)
    self.nc.all_engine_barrier()
    popped = self.nc._tile_sem_poison_stack.pop()
    assert popped is self._sem_poison
    self.nc.clear_and_free_semaphores(list(self.sems.allocated().values()))
    self.nc.all_engine_barrier()


tile.TileContext._drain_and_barrier = _patched_drain_and_barrier

# ------------------------------------------------------------- ntff hook
# The env's antenv package lacks axon_hooks; recreate the NTFF profile hook
# via ctypes so run_bass_kernel_spmd(trace=True) works (test/profiling only).
_AXON_SO = "/opt/axon/libaxon_pjrt.so"


def _install_ntff_hook():
    if "antenv.axon_hooks" in sys.modules:
        return
    try:
        lib = ctypes.CDLL(_AXON_SO)
        lib.axon_start_nrt_profile.argtypes = [
            ctypes.POINTER(ctypes.c_int64),
            ctypes.c_size_t,
        ]
        lib.axon_start_nrt_profile.restype = ctypes.c_int64
        lib.axon_stop_nrt_profile.argtypes = [ctypes.c_char_p]
        lib.axon_stop_nrt_profile.restype = ctypes.c_int64
    except Exception:
        return

    @contextlib.contextmanager
    def _hook(output_dir, device_ids):
        import jax

        jax.devices()
        if device_ids:
            ids = (ctypes.c_int64 * len(device_ids))(*device_ids)
            rc = lib.axon_start_nrt_profile(ids, len(device_ids))
        else:
            rc = lib.axon_start_nrt_profile(None, 0)
        if rc != 0:
            raise RuntimeError(f"axon_start_nrt_profile rc={rc}")
        try:
            yield
        finally:
            n = lib.axon_stop_nrt_profile(str(output_dir).encode())
            print(f"profile: {n} file(s) written to {output_dir}", file=sys.stderr)

    mod = types.ModuleType("antenv.axon_hooks")
    mod.get_axon_ntff_profile_hook = lambda: _hook
    mod.set_axon_ntff_profile_hook = lambda h: None
    sys.modules["antenv.axon_hooks"] = mod


# ---------------------------------------------------------------- host prep


def _build_plan(row, col, pos, n, tau=1e-2, knear=8, min_class_slots=128 * 1024):
    """Split edges into far (bf16-safe, dist^2 >= tau) and near (exact fp32
    section, <= knear per node; overflow demoted to far). Returns plan with
    per-class far geometry + near geometry."""
    deg = np.bincount(row, minlength=n).astype(np.int64)
    order = np.argsort(row, kind="stable")
    row_s = row[order]
    col_s = col[order]
    pd = pos[col_s].astype(np.float32) - pos[row_s].astype(np.float32)
    r2e = (pd * pd).sum(1)
    near_e = r2e < np.float32(tau)
    # within each node's run, put near edges first (stable)
    order2 = np.argsort(row_s * 2 + (~near_e), kind="stable")
    col_s2 = col_s[order2]
    offs = np.zeros(n + 1, dtype=np.int64)
    np.cumsum(deg, out=offs[1:])
    dnear = np.bincount(row_s[near_e], minlength=n)
    dnear = np.minimum(dnear, knear).astype(np.int64)
    dfar = deg - dnear
    affected = dnear > 0

    active = deg > 0
    kraw = ((np.maximum(dfar, 1) + 3) // 4) * 4
    uniq = np.unique(kraw[active])
    groups = []
    pend = []
    pend_slots = 0
    for K in uniq:
        ids = np.nonzero(active & (kraw == K))[0]
        pend.append(ids)
        pend_slots += ids.size * int(K)
        if pend_slots >= min_class_slots or K == uniq[-1]:
            allids = np.concatenate(pend)
            groups.append((int(K), allids))
            pend = []
            pend_slots = 0

    plan = []
    for K, ids in groups:
        aff = affected[ids]
        # affected first (stable), then round-robin core split
        ids = ids[np.argsort(~aff, kind="stable")]
        per_core = -(-ids.size // N_CORES)
        m = -(-per_core // P) * P  # per-core nodes, padded to mult of 128
        a_max = 0
        for c in range(N_CORES):
            a_max = max(a_max, int(affected[ids[c::N_CORES]].sum()))
        w = -(-a_max // P)  # near cols per class (common)
        plan.append((K, ids, m, w))
    return plan, deg, dnear, dfar, offs, col_s2, knear


def _build_streams(plan, deg, dnear, dfar, offs, col_s2, knear, nodedata):
    """nodedata: [n,7] f32. Builds per-core far bf16 streams (interleaved
    node->partition mapping) + near fp32 section. Returns cores, NN, S, Wq."""
    import ml_dtypes

    bf = ml_dtypes.bfloat16
    S = sum(m * K for K, _, m, _ in plan)
    NN = sum(m for _, _, m, _ in plan)
    Wq = sum(w for _, _, _, w in plan)
    cores = []
    for c in range(N_CORES):
        colp = np.zeros((7, P, S // P), bf)
        nodb = np.zeros((7, P, NN // P), bf)
        cnt = np.ones((P, NN // P), np.float32)
        nearA = np.zeros((7, P, Wq * knear), np.float32)
        nearB = np.zeros((7, P, Wq * knear), np.float32)
        off_slots = 0
        off_nodes = 0
        off_q = 0
        for K, ids, m, w in plan:
            q = m // P
            ids_c = ids[c::N_CORES]
            k_real = ids_c.size
            # ---- far stream ----
            vals = np.zeros((m, K, 7), np.float32)
            nodevals = np.zeros((m, 7), np.float32)
            cv = np.ones(m, np.float32)
            if k_real > 0:
                colmat = np.empty((k_real, K), np.int64)
                colmat[:] = ids_c[:, None]
                df = dfar[ids_c]
                oo = offs[ids_c] + dnear[ids_c]  # far edges after near ones
                ar = np.arange(K)[None, :]
                valid = ar < df[:, None]
                src_idx = (oo[:, None] + ar)[valid]
                colmat[valid] = col_s2[src_idx]
                vals[:k_real] = nodedata[colmat]
                nodevals[:k_real] = nodedata[ids_c]
                cv[:k_real] = np.maximum(deg[ids_c], 1).astype(np.float32)
            wv = vals.reshape(q, P, K, 7).transpose(3, 1, 0, 2).reshape(7, P, q * K)
            colp[:, :, off_slots : off_slots + q * K] = wv.astype(bf)
            nodb[:, :, off_nodes : off_nodes + q] = (
                nodevals.reshape(q, P, 7).transpose(2, 1, 0).astype(bf)
            )
            cnt[:, off_nodes : off_nodes + q] = cv.reshape(q, P).T
            # ---- near section (affected nodes = first a of ids_c) ----
            if w > 0:
                mw = w * P
                a = int((dnear[ids_c] > 0).sum()) if k_real > 0 else 0
                nvals = np.zeros((mw, knear, 7), np.float32)
                nnode = np.zeros((mw, knear, 7), np.float32)
                if a > 0:
                    aid = ids_c[:a]
                    colm = np.empty((a, knear), np.int64)
                    colm[:] = aid[:, None]
                    dn = dnear[aid]
                    oo = offs[aid]
                    ar = np.arange(knear)[None, :]
                    valid = ar < dn[:, None]
                    src_idx = (oo[:, None] + ar)[valid]
                    colm[valid] = col_s2[src_idx]
                    nvals[:a] = nodedata[colm]
                    nnode[:a] = nodedata[aid][:, None, :]
                wa = nvals.reshape(w, P, knear, 7).transpose(3, 1, 0, 2)
                wb = nnode.reshape(w, P, knear, 7).transpose(3, 1, 0, 2)
                nearA[:, :, off_q * knear : (off_q + w) * knear] = wa.reshape(
                    7, P, w * knear
                )
                nearB[:, :, off_q * knear : (off_q + w) * knear] = wb.reshape(
                    7, P, w * knear
                )
            off_slots += q * K
            off_nodes += q
            off_q += w
        cores.append(dict(colp=colp, nodb=nodb, cnt=cnt, nearA=nearA, nearB=nearB))
    return cores, NN, S, Wq


# ---------------------------------------------------------------- bass build


def _class_tiles(plan):
    """Yield (K, g_nodes_in_tile, slot_col_offset, node_col_offset) splits."""
    tiles = []
    off_s = 0
    off_n = 0
    for K, _, m, _ in plan:
        q = m // P
        gmax = max(1, F_TILE // K)
        i = 0
        while i < q:
            g = min(gmax, q - i)
            tiles.append((K, g, off_s + i * K, off_n + i))
            i += g
        off_s += q * K
        off_n += q
    return tiles


def _class_merges(plan):
    """Per-class (node_col_offset, near_col_offset, w) for near merges."""
    merges = []
    off_n = 0
    off_q = 0
    for K, _, m, w in plan:
        if w > 0:
            merges.append((off_n, off_q, w))
        off_n += m // P
        off_q += w
    return merges


def _raw_scalar_act(nc, out, in_, func, bias=0.0, scale=1.0):
    """InstActivation without the python wrapper's Reciprocal ban and without
    the const-AP bias conversion (immediates work for these funcs here)."""
    inputs = [nc.scalar.lower_ap(in_)]
    for arg in (bias, scale, 0.0):
        inputs.append(mybir.ImmediateValue(dtype=mybir.dt.float32, value=arg))
    return nc.scalar.add_instruction(
        mybir.InstActivation(
            name=nc.get_next_instruction_name(),
            func=func,
            ins=inputs,
            outs=[nc.scalar.lower_ap(out)],
        )
    )


def _build_nc(plan, NN, S, Wq, KN, DQ):
    """SPMD bass program: bf16 far pipeline + small exact fp32 near section."""
    fp32 = mybir.dt.float32
    bf16 = mybir.dt.bfloat16
    nc = bass.Bass("TRN2", target_bir_lowering=False)
    W = S // P
    Q = NN // P
    WN = max(Wq, 1) * KN

    cols = [
        nc.dram_tensor(f"col{i}", [P, W], bf16, kind="ExternalInput")
        for i in range(7)
    ]
    nodb = nc.dram_tensor("nodb", [P, 7 * Q], bf16, kind="ExternalInput")
    cntT = nc.dram_tensor("cnt", [P, Q], fp32, kind="ExternalInput")
    nearA = nc.dram_tensor("nearA", [P, 7 * WN], fp32, kind="ExternalInput")
    nearB = nc.dram_tensor("nearB", [P, 7 * WN], fp32, kind="ExternalInput")
    dlp = nc.dram_tensor("dlp", [P, 4 * DQ], fp32, kind="ExternalInput")
    dlt = nc.dram_tensor("dlt", [P, 4 * DQ], fp32, kind="ExternalInput")
    out = nc.dram_tensor("out", [P, 8], fp32, kind="ExternalOutput")

    AF = mybir.ActivationFunctionType
    OP = mybir.AluOpType

    with tile.TileContext(nc) as tc:
        with (
            tc.tile_pool(name="resident", bufs=1) as res_pool,
            tc.tile_pool(name="colp", bufs=2) as col_pool,
            tc.tile_pool(name="exp", bufs=2) as exp_pool,
            tc.tile_pool(name="tmp", bufs=2) as tmp_pool,
            tc.tile_pool(name="tmp1", bufs=1) as tmp1_pool,
        ):
            nodbt = res_pool.tile([P, 7 * Q], bf16)
            nc.sync.dma_start(nodbt[:], nodb.ap()[:])
            cntt = res_pool.tile([P, Q], fp32)
            nc.sync.dma_start(cntt[:], cntT.ap()[:])
            acc = res_pool.tile([P, 7 * Q], fp32)  # vg, lu, lv, lw, px, py, pz

            # ---- data loss (small, fp32) ----
            dlpt = res_pool.tile([P, 4 * DQ], fp32)
            nc.sync.dma_start(dlpt[:], dlp.ap()[:])
            dltt = res_pool.tile([P, 4 * DQ], fp32)
            nc.sync.dma_start(dltt[:], dlt.ap()[:])
            dld = res_pool.tile([P, 4 * DQ], fp32)
            nc.vector.tensor_sub(dld[:], dlpt[:], dltt[:])
            acc_vel = res_pool.tile([P, 1], fp32)
            acc_pres = res_pool.tile([P, 1], fp32)
            nc.scalar.activation(
                dld[:, 0 : 3 * DQ], dld[:, 0 : 3 * DQ], AF.Square,
                accum_out=acc_vel[:],
            )
            nc.scalar.activation(
                dld[:, 3 * DQ : 4 * DQ], dld[:, 3 * DQ : 4 * DQ], AF.Square,
                accum_out=acc_pres[:],
            )

            # ---- far loop (all bf16) ----
            acc4 = acc[:].rearrange("p (i q) -> p i q", i=7, q=Q)
            for K, g, off_s, off_n in _class_tiles(plan):
                F = g * K
                cvt = col_pool.tile([P, 7 * F], bf16, tag="cvt", name="cvt")
                for i in range(7):
                    nc.sync.dma_start(
                        cvt[:, i * F : (i + 1) * F],
                        cols[i].ap()[:, off_s : off_s + F],
                    )

                # node expand, all 7 planes in one ScalarE op
                evt = exp_pool.tile([P, 7 * F], bf16, tag="evt", name="evt")
                a7 = nodbt[:].rearrange("p (i q) -> p i q", i=7, q=Q)
                nc.scalar.copy(
                    evt[:].rearrange("p (i g k) -> p i g k", i=7, g=g, k=K),
                    a7[:, :, off_n : off_n + g]
                    .unsqueeze(-1)
                    .broadcast_to([P, 7, g, K]),
                )

                # all 7 diffs in one bf16 sub (2x)
                dft = tmp1_pool.tile([P, 7 * F], bf16, tag="dft", name="dft")
                nc.vector.tensor_sub(dft[:], cvt[:], evt[:])

                # r2 = dx^2+dy^2+dz^2 in bf16
                sqt = tmp_pool.tile([P, 3 * F], bf16, tag="sqt", name="sqt")
                nc.vector.tensor_mul(sqt[:], dft[:, 0 : 3 * F], dft[:, 0 : 3 * F])
                nc.vector.tensor_add(sqt[:, 0:F], sqt[:, 0:F], sqt[:, F : 2 * F])
                nc.vector.tensor_add(
                    sqt[:, 0:F], sqt[:, 0:F], sqt[:, 2 * F : 3 * F]
                )
                r2 = sqt[:, 0:F]

                # d1 = 1/(sqrt(r2)+eps), d2 = 1/(r2+eps) via ScalarE LUTs
                ss = tmp_pool.tile([P, F], bf16, tag="ss", name="ss")
                nc.scalar.activation(ss[:], r2, AF.Sqrt)
                d1 = tmp_pool.tile([P, F], bf16, tag="d1")
                _raw_scalar_act(nc, d1[:], ss[:], AF.Reciprocal, bias=EPS)
                d2 = tmp_pool.tile([P, F], bf16, tag="d2")
                _raw_scalar_act(nc, d2[:], r2, AF.Reciprocal, bias=EPS)

                vals = tmp1_pool.tile([P, 7 * F], bf16, tag="vals", name="vals")
                # g = sum duvw*dxyz ; velgrad = g*d1 -> vals plane 0
                prt = tmp1_pool.tile([P, 3 * F], bf16, tag="prt", name="prt")
                nc.vector.tensor_mul(
                    prt[:], dft[:, 3 * F : 6 * F], dft[:, 0 : 3 * F]
                )
                nc.vector.tensor_add(prt[:, 0:F], prt[:, 0:F], prt[:, F : 2 * F])
                nc.vector.tensor_add(
                    prt[:, 0:F], prt[:, 0:F], prt[:, 2 * F : 3 * F]
                )
                nc.vector.tensor_mul(vals[:, 0:F], prt[:, 0:F], d1[:])

                # cp = dq*d1*d1 (in-place in dft plane 6)
                cpq = dft[:, 6 * F : 7 * F]
                nc.vector.tensor_mul(cpq, cpq, d1[:])
                nc.vector.tensor_mul(cpq, cpq, d1[:])

                # pg = cp * dxyz -> vals planes 4:7 (cp broadcast over 3)
                cp3 = bass.AP(dft[:].tensor, dft[:].offset + 6 * F,
                              [dft[:].ap[0], [0, 3], [1, F]])
                nc.vector.tensor_mul(vals[:, 4 * F : 7 * F].rearrange(
                    "p (i f) -> p i f", i=3, f=F), cp3,
                    dft[:, 0 : 3 * F].rearrange("p (i f) -> p i f", i=3, f=F))
                # lap = duvw * d2 -> vals planes 1:4 (d2 broadcast over 3)
                d23 = bass.AP(d2[:].tensor, d2[:].offset,
                              [d2[:].ap[0], [0, 3], [1, F]])
                nc.vector.tensor_mul(vals[:, 1 * F : 4 * F].rearrange(
                    "p (i f) -> p i f", i=3, f=F),
                    dft[:, 3 * F : 6 * F].rearrange("p (i f) -> p i f", i=3, f=F),
                    d23)

                # segment sums: aligned halving tree over K down to 2
                v4 = vals[:].rearrange("p (i g k) -> p i g k", i=7, g=g, k=K)
                k = K
                while k > 2:
                    if k % 4 == 2:
                        nc.vector.tensor_add(
                            v4[:, :, :, 0:2], v4[:, :, :, 0:2],
                            v4[:, :, :, k - 2 : k],
                        )
                        k -= 2
                    else:
                        h = k // 2
                        nc.vector.tensor_add(
                            v4[:, :, :, 0:h], v4[:, :, :, 0:h],
                            v4[:, :, :, h:k],
                        )
                        k = h
                nc.vector.tensor_reduce(
                    acc4[:, :, off_n : off_n + g],
                    v4[:, :, :, 0:k],
                    mybir.AxisListType.X,
                    OP.add,
                )

            # ---- near section (exact fp32, tiny) ----
            if Wq > 0:
                nat = col_pool.tile([P, 7 * WN], fp32, tag="cvt", name="nat")
                nc.sync.dma_start(nat[:], nearA.ap()[:])
                nbt = exp_pool.tile([P, 7 * WN], fp32, tag="evt", name="nbt")
                nc.sync.dma_start(nbt[:], nearB.ap()[:])
                nd = tmp1_pool.tile([P, 7 * WN], fp32, tag="dft", name="nd")
                nc.vector.tensor_sub(nd[:], nat[:], nbt[:])
                nsq = tmp_pool.tile([P, 3 * WN], fp32, tag="sqt", name="nsq")
                nc.scalar.activation(nsq[:], nd[:, 0 : 3 * WN], AF.Square)
                nc.vector.tensor_add(
                    nsq[:, 0:WN], nsq[:, 0:WN], nsq[:, WN : 2 * WN]
                )
                nc.vector.tensor_add(
                    nsq[:, 0:WN], nsq[:, 0:WN], nsq[:, 2 * WN : 3 * WN]
                )
                r2n = nsq[:, 0:WN]
                ssn = tmp_pool.tile([P, WN], fp32, tag="ss", name="ssn")
                nc.scalar.activation(ssn[:], r2n, AF.Sqrt)
                d1n = tmp_pool.tile([P, WN], fp32, tag="d1", name="d1n")
                _raw_scalar_act(nc, d1n[:], ssn[:], AF.Reciprocal, bias=EPS)
                d2n = tmp_pool.tile([P, WN], fp32, tag="d2", name="d2n")
                _raw_scalar_act(nc, d2n[:], r2n, AF.Reciprocal, bias=EPS)

                nvals = tmp1_pool.tile([P, 7 * WN], fp32, tag="vals", name="nvals")
                prn = tmp1_pool.tile([P, 3 * WN], fp32, tag="prt", name="prn")
                nc.vector.tensor_mul(
                    prn[:], nd[:, 3 * WN : 6 * WN], nd[:, 0 : 3 * WN]
                )
                nc.vector.tensor_add(
                    prn[:, 0:WN], prn[:, 0:WN], prn[:, WN : 2 * WN]
                )
                nc.vector.tensor_add(
                    prn[:, 0:WN], prn[:, 0:WN], prn[:, 2 * WN : 3 * WN]
                )
                nc.vector.tensor_mul(nvals[:, 0:WN], prn[:, 0:WN], d1n[:])
                cpn = nd[:, 6 * WN : 7 * WN]
                nc.vector.tensor_mul(cpn, cpn, d1n[:])
                nc.vector.tensor_mul(cpn, cpn, d1n[:])
                cp3n = bass.AP(nd[:].tensor, nd[:].offset + 6 * WN,
                               [nd[:].ap[0], [0, 3], [1, WN]])
                nc.vector.tensor_mul(
                    nvals[:, 4 * WN : 7 * WN].rearrange(
                        "p (i f) -> p i f", i=3, f=WN),
                    cp3n,
                    nd[:, 0 : 3 * WN].rearrange("p (i f) -> p i f", i=3, f=WN))
                d23n = bass.AP(d2n[:].tensor, d2n[:].offset,
                               [d2n[:].ap[0], [0, 3], [1, WN]])
                nc.vector.tensor_mul(
                    nvals[:, 1 * WN : 4 * WN].rearrange(
                        "p (i f) -> p i f", i=3, f=WN),
                    nd[:, 3 * WN : 6 * WN].rearrange("p (i f) -> p i f", i=3, f=WN),
                    d23n)

                accN = res_pool.tile([P, 7 * Wq], fp32)
                nv4 = nvals[:].rearrange(
                    "p (i w k) -> p i w k", i=7, w=Wq, k=KN
                )
                k = KN
                while k > 2:
                    h = k // 2
                    nc.vector.tensor_add(
                        nv4[:, :, :, 0:h], nv4[:, :, :, 0:h], nv4[:, :, :, h:k]
                    )
                    k = h
                accN4 = accN[:].rearrange("p (i w) -> p i w", i=7, w=Wq)
                nc.vector.tensor_reduce(
                    accN4, nv4[:, :, :, 0:k], mybir.AxisListType.X, OP.add
                )
                # merge near partial sums into acc (aligned contiguous ranges)
                for off_n, off_q, w in _class_merges(plan):
                    nc.vector.tensor_add(
                        acc4[:, :, off_n : off_n + w],
                        acc4[:, :, off_n : off_n + w],
                        accN4[:, :, off_q : off_q + w],
                    )

            # ---- finish (fp32, small) ----
            icnt = res_pool.tile([P, Q], fp32)
            _raw_scalar_act(nc, icnt[:], cntt[:], AF.Reciprocal)
            div = res_pool.tile([P, Q], fp32)
            nc.vector.tensor_mul(div[:], acc[:, 0:Q], icnt[:])
            acc_div2 = res_pool.tile([P, 1], fp32)
            nc.scalar.activation(div[:], div[:], AF.Square, accum_out=acc_div2[:])
            acc_m = [
                res_pool.tile([P, 1], fp32, tag=f"am{i}", name=f"am{i}")
                for i in range(3)
            ]
            for i in range(3):
                r = res_pool.tile([P, Q], fp32, tag="rfin")
                nc.vector.scalar_tensor_tensor(
                    r[:],
                    acc[:, (1 + i) * Q : (2 + i) * Q],
                    1.0 / REYNOLDS,
                    acc[:, (4 + i) * Q : (5 + i) * Q],
                    OP.mult,
                    OP.add,
                )
                nc.vector.tensor_mul(r[:], r[:], icnt[:])
                nc.scalar.activation(r[:], r[:], AF.Square, accum_out=acc_m[i][:])

            outt = res_pool.tile([P, 8], fp32)
            nc.vector.memset(outt[:], 0.0)
            nc.vector.tensor_copy(outt[:, 0:1], acc_vel[:])
            nc.vector.tensor_copy(outt[:, 1:2], acc_pres[:])
            nc.vector.tensor_copy(outt[:, 2:3], acc_div2[:])
            nc.vector.tensor_copy(outt[:, 3:4], acc_m[0][:])
            nc.vector.tensor_copy(outt[:, 4:5], acc_m[1][:])
            nc.vector.tensor_copy(outt[:, 5:6], acc_m[2][:])
            nc.sync.dma_start(out.ap()[:], outt[:])

    return nc


# ---------------------------------------------------------------- entry

_CACHE = {}


def _get_nc(key, plan, NN, S, Wq, KN, DQ):
    if key not in _CACHE:
        _CACHE[key] = _build_nc(plan, NN, S, Wq, KN, DQ)
    return _CACHE[key]


LAST_RESULT = None  # BassKernelResults of the most recent run (for profiling)


def kernel(pred, target, edge_index, pos, _trace_dir=None):
    global LAST_RESULT
    pred = np.asarray(pred)
    target = np.asarray(target)
    pos = np.asarray(pos).astype(np.float32)
    row = np.asarray(edge_index[0]).astype(np.int64)
    col = np.asarray(edge_index[1]).astype(np.int64)
    n = pred.shape[0]

    plan, deg, dnear, dfar, offs, col_s2, KN = _build_plan(row, col, pos, n)
    nodedata = np.concatenate(
        [pos, pred.astype(np.float32)], axis=1
    )
    cores, NN, S, Wq = _build_streams(
        plan, deg, dnear, dfar, offs, col_s2, KN, nodedata
    )
    Qn = NN // P
    WN = max(Wq, 1) * KN

    # data-loss slices: split all n nodes across cores, pad to mult of 128
    per = -(-n // N_CORES)
    DQ = -(-per // P)
    predf = pred.astype(np.float32)
    targf = target.astype(np.float32)

    in_maps = []
    for c in range(N_CORES):
        lo, hi = c * per, min((c + 1) * per, n)
        ps = np.zeros((P * DQ, 4), np.float32)
        ts = np.zeros((P * DQ, 4), np.float32)
        ps[: hi - lo] = predf[lo:hi]
        ts[: hi - lo] = targf[lo:hi]
        # [P, 4*DQ] with plane-major columns: plane i at cols [i*DQ, (i+1)*DQ)
        dlp = ps.reshape(P, DQ, 4).transpose(0, 2, 1).reshape(P, 4 * DQ)
        dlt = ts.reshape(P, DQ, 4).transpose(0, 2, 1).reshape(P, 4 * DQ)
        cc = cores[c]
        m = dict(
            cnt=np.ascontiguousarray(cc["cnt"]),
            nodb=np.ascontiguousarray(
                cc["nodb"].transpose(1, 0, 2).reshape(P, 7 * Qn)
            ),
            nearA=np.ascontiguousarray(
                cc["nearA"].transpose(1, 0, 2).reshape(P, 7 * (Wq * KN))
            ) if Wq > 0 else np.zeros((P, 7 * WN), np.float32),
            nearB=np.ascontiguousarray(
                cc["nearB"].transpose(1, 0, 2).reshape(P, 7 * (Wq * KN))
            ) if Wq > 0 else np.zeros((P, 7 * WN), np.float32),
            dlp=np.ascontiguousarray(dlp),
            dlt=np.ascontiguousarray(dlt),
        )
        for i in range(7):
            m[f"col{i}"] = np.ascontiguousarray(cc["colp"][i])
        in_maps.append(m)

    key = (tuple((K, mm, w) for K, _, mm, w in plan), NN, S, Wq, KN, DQ)
    plan_geom = [(K, None, mm, w) for K, _, mm, w in plan]
    nc = _get_nc(key, plan_geom, NN, S, Wq, KN, DQ)

    if _trace_dir is not None:
        _install_ntff_hook()
        res = run_bass_kernel_spmd(
            nc, in_maps, core_ids=list(range(N_CORES)), trace=True,
            tmpdir=_trace_dir,
        )
    else:
        res = run_bass_kernel_spmd(nc, in_maps, core_ids=list(range(N_CORES)))
    LAST_RESULT = res

    tot = np.zeros(8, np.float64)
    for c in range(N_CORES):
        tot += res.results[c]["out"].astype(np.float64).sum(axis=0)
    s_vel, s_pres, s_div2, am0, am1, am2 = (
        tot[0], tot[1], tot[2], tot[3], tot[4], tot[5]
    )
    loss = (
        s_vel / (3 * n)
        + s_pres / n
        + LAMBDA_CONT * s_div2 / n
        + LAMBDA_MOM * (am0 + am1 + am2) / (3 * n)
    )
    return np.float32(loss)


# revision 36
# speedup vs baseline: 1.1670x; 1.0701x over previous
"""PhysicsInformedLoss on 8 Trainium2 NeuronCores.

Sharding strategy (degree-class padded CSR):
- Edges are grouped by destination node `row` (the scatter target of every
  segment-mean in the reference). Nodes with deg>0 are binned into degree
  classes K (multiples of 4, small classes merged); each node gets exactly K
  contiguous "slots" (its edges + self-pads, pads contribute exactly 0).
- Nodes of each class are split evenly across the 8 cores (identical padded
  per-core counts -> one SPMD program). Per core, node i of a class maps to
  partition p = i // q (q nodes/partition), so every per-node segment sum is
  a static strided reduction along the free dimension.
- The host gathers the 7 col-side planes (pos xyz, vel uvw, p) in slot order
  (this is the "shard the edges" data layout step); the row side is the
  per-node resident plane broadcast along K by a stride-0 access pattern.
- Device per core: stream col planes, compute all per-edge terms, strided
  per-node reduce, finish div/residual/squares; output per-partition partial
  sums. Host sums 8x128 partials and forms the scalar loss.
"""
import contextlib
import ctypes
import os
import sys
import tempfile
import types

import numpy as np

import concourse.bass as bass
import concourse.tile as tile
from concourse import mybir
from concourse.vector_clock import ScopedClock
from concourse.bass_utils import run_bass_kernel_spmd

N_CORES = 8
P = 128
EPS = 1e-8
REYNOLDS = 1000000.0
LAMBDA_CONT = 0.1
LAMBDA_MOM = 0.01
F_TILE = 1408  # target per-partition columns per tile

# ---------------------------------------------------------------- tile patch
# walrus in this environment allows only ONE sync-wait per instruction, but
# Tile's scheduler can emit several. Split surplus waits onto engine NOPs
# inserted right before the offending instruction.
_MAX_WAITS = 1


def _split_multi_waits(nc, handles):
    work = []
    for fn in nc.m.functions:
        for bb in fn.blocks:
            items = []
            for inst in bb.instructions:
                si = inst.sync_info
                waits = list(si.on_wait) if si and si.on_wait else []
                if len(waits) > _MAX_WAITS:
                    keep = len(waits) - _MAX_WAITS
                    extra = waits[:keep]
                    si.on_wait = waits[keep:]
                    chunks = [
                        extra[i : i + _MAX_WAITS]
                        for i in range(0, len(extra), _MAX_WAITS)
                    ]
                    items.append((inst.name, inst.engine, chunks))
            if items:
                work.append((bb, items))
    if not work:
        return
    created = {}
    placements = {}
    for bb, items in work:
        plc = {}
        for inst_name, engine, chunks in items:
            nops = []
            for chunk in chunks:
                for w in chunk:
                    h = handles.get(w.ant_name)
                    assert h is not None, f"no sem handle for {w.ant_name}"
                    ni = nc.engines[engine].wait_ge(h, w.wait_value)
                    created[ni.ins.name] = None
                    nops.append(ni.ins)
            plc[inst_name] = nops
        placements[id(bb)] = plc
    for fn in nc.m.functions:
        for bb in fn.blocks:
            plc = placements.get(id(bb), {})
            newlist = []
            for inst in bb.instructions:
                if inst.name in created:
                    continue
                if inst.name in plc:
                    newlist.extend(plc[inst.name])
                newlist.append(inst)
            bb.instructions = newlist


def _patched_drain_and_barrier(self, tick_clock, wait_clock):
    drain_inst = self.nc.sync.drain()
    wait_clock.add_sem_waits(
        drain_inst.ins, ScopedClock({None: tick_clock.global_clock})
    )
    handles = {h.name: h for h in self.sems.allocated().values()}
    _split_multi_waits(self.nc, handles)
    self.nc.all_engine_barrier()
    popped = self.nc._tile_sem_poison_stack.pop()
    assert popped is self._sem_poison
    self.nc.clear_and_free_semaphores(list(self.sems.allocated().values()))
    self.nc.all_engine_barrier()


tile.TileContext._drain_and_barrier = _patched_drain_and_barrier

# ------------------------------------------------------------- ntff hook
# The env's antenv package lacks axon_hooks; recreate the NTFF profile hook
# via ctypes so run_bass_kernel_spmd(trace=True) works (test/profiling only).
_AXON_SO = "/opt/axon/libaxon_pjrt.so"


def _install_ntff_hook():
    if "antenv.axon_hooks" in sys.modules:
        return
    try:
        lib = ctypes.CDLL(_AXON_SO)
        lib.axon_start_nrt_profile.argtypes = [
            ctypes.POINTER(ctypes.c_int64),
            ctypes.c_size_t,
        ]
        lib.axon_start_nrt_profile.restype = ctypes.c_int64
        lib.axon_stop_nrt_profile.argtypes = [ctypes.c_char_p]
        lib.axon_stop_nrt_profile.restype = ctypes.c_int64
    except Exception:
        return

    @contextlib.contextmanager
    def _hook(output_dir, device_ids):
        import jax

        jax.devices()
        if device_ids:
            ids = (ctypes.c_int64 * len(device_ids))(*device_ids)
            rc = lib.axon_start_nrt_profile(ids, len(device_ids))
        else:
            rc = lib.axon_start_nrt_profile(None, 0)
        if rc != 0:
            raise RuntimeError(f"axon_start_nrt_profile rc={rc}")
        try:
            yield
        finally:
            n = lib.axon_stop_nrt_profile(str(output_dir).encode())
            print(f"profile: {n} file(s) written to {output_dir}", file=sys.stderr)

    mod = types.ModuleType("antenv.axon_hooks")
    mod.get_axon_ntff_profile_hook = lambda: _hook
    mod.set_axon_ntff_profile_hook = lambda h: None
    sys.modules["antenv.axon_hooks"] = mod


# ---------------------------------------------------------------- host prep


def _build_plan(row, col, pos, n, tau=1e-2, knear=8, min_class_slots=128 * 1024):
    """Split edges into far (bf16-safe, dist^2 >= tau) and near (exact fp32
    section, <= knear per node; overflow demoted to far). Returns plan with
    per-class far geometry + near geometry."""
    deg = np.bincount(row, minlength=n).astype(np.int64)
    order = np.argsort(row, kind="stable")
    row_s = row[order]
    col_s = col[order]
    pd = pos[col_s].astype(np.float32) - pos[row_s].astype(np.float32)
    r2e = (pd * pd).sum(1)
    near_e = r2e < np.float32(tau)
    # within each node's run, put near edges first (stable)
    order2 = np.argsort(row_s * 2 + (~near_e), kind="stable")
    col_s2 = col_s[order2]
    offs = np.zeros(n + 1, dtype=np.int64)
    np.cumsum(deg, out=offs[1:])
    dnear = np.bincount(row_s[near_e], minlength=n)
    dnear = np.minimum(dnear, knear).astype(np.int64)
    dfar = deg - dnear
    affected = dnear > 0

    active = deg > 0
    kraw = ((np.maximum(dfar, 1) + 3) // 4) * 4
    uniq = np.unique(kraw[active])
    groups = []
    pend = []
    pend_slots = 0
    for K in uniq:
        ids = np.nonzero(active & (kraw == K))[0]
        pend.append(ids)
        pend_slots += ids.size * int(K)
        if pend_slots >= min_class_slots or K == uniq[-1]:
            allids = np.concatenate(pend)
            groups.append((int(K), allids))
            pend = []
            pend_slots = 0

    plan = []
    for K, ids in groups:
        aff = affected[ids]
        # affected first (stable), then round-robin core split
        ids = ids[np.argsort(~aff, kind="stable")]
        per_core = -(-ids.size // N_CORES)
        m = -(-per_core // P) * P  # per-core nodes, padded to mult of 128
        a_max = 0
        for c in range(N_CORES):
            a_max = max(a_max, int(affected[ids[c::N_CORES]].sum()))
        w = -(-a_max // P)  # near cols per class (common)
        plan.append((K, ids, m, w))
    return plan, deg, dnear, dfar, offs, col_s2, knear


def _build_streams(plan, deg, dnear, dfar, offs, col_s2, knear, nodedata):
    """nodedata: [n,7] f32. Builds per-core far bf16 streams (interleaved
    node->partition mapping) + near fp32 section. Returns cores, NN, S, Wq."""
    import ml_dtypes

    bf = ml_dtypes.bfloat16
    S = sum(m * K for K, _, m, _ in plan)
    NN = sum(m for _, _, m, _ in plan)
    Wq = sum(w for _, _, _, w in plan)
    cores = []
    for c in range(N_CORES):
        colp = np.zeros((7, P, S // P), bf)
        nodb = np.zeros((7, P, NN // P), bf)
        cnt = np.ones((P, NN // P), np.float32)
        nearA = np.zeros((7, P, Wq * knear), np.float32)
        nearB = np.zeros((7, P, Wq * knear), np.float32)
        off_slots = 0
        off_nodes = 0
        off_q = 0
        for K, ids, m, w in plan:
            q = m // P
            ids_c = ids[c::N_CORES]
            k_real = ids_c.size
            # ---- far stream ----
            vals = np.zeros((m, K, 7), np.float32)
            nodevals = np.zeros((m, 7), np.float32)
            cv = np.ones(m, np.float32)
            if k_real > 0:
                colmat = np.empty((k_real, K), np.int64)
                colmat[:] = ids_c[:, None]
                df = dfar[ids_c]
                oo = offs[ids_c] + dnear[ids_c]  # far edges after near ones
                ar = np.arange(K)[None, :]
                valid = ar < df[:, None]
                src_idx = (oo[:, None] + ar)[valid]
                colmat[valid] = col_s2[src_idx]
                vals[:k_real] = nodedata[colmat]
                nodevals[:k_real] = nodedata[ids_c]
                cv[:k_real] = np.maximum(deg[ids_c], 1).astype(np.float32)
            wv = vals.reshape(q, P, K, 7).transpose(3, 1, 0, 2).reshape(7, P, q * K)
            colp[:, :, off_slots : off_slots + q * K] = wv.astype(bf)
            nodb[:, :, off_nodes : off_nodes + q] = (
                nodevals.reshape(q, P, 7).transpose(2, 1, 0).astype(bf)
            )
            cnt[:, off_nodes : off_nodes + q] = cv.reshape(q, P).T
            # ---- near section (affected nodes = first a of ids_c) ----
            if w > 0:
                mw = w * P
                a = int((dnear[ids_c] > 0).sum()) if k_real > 0 else 0
                nvals = np.zeros((mw, knear, 7), np.float32)
                nnode = np.zeros((mw, knear, 7), np.float32)
                if a > 0:
                    aid = ids_c[:a]
                    colm = np.empty((a, knear), np.int64)
                    colm[:] = aid[:, None]
                    dn = dnear[aid]
                    oo = offs[aid]
                    ar = np.arange(knear)[None, :]
                    valid = ar < dn[:, None]
                    src_idx = (oo[:, None] + ar)[valid]
                    colm[valid] = col_s2[src_idx]
                    nvals[:a] = nodedata[colm]
                    nnode[:a] = nodedata[aid][:, None, :]
                wa = nvals.reshape(w, P, knear, 7).transpose(3, 1, 0, 2)
                wb = nnode.reshape(w, P, knear, 7).transpose(3, 1, 0, 2)
                nearA[:, :, off_q * knear : (off_q + w) * knear] = wa.reshape(
                    7, P, w * knear
                )
                nearB[:, :, off_q * knear : (off_q + w) * knear] = wb.reshape(
                    7, P, w * knear
                )
            off_slots += q * K
            off_nodes += q
            off_q += w
        cores.append(dict(colp=colp, nodb=nodb, cnt=cnt, nearA=nearA, nearB=nearB))
    return cores, NN, S, Wq


# ---------------------------------------------------------------- bass build


def _class_tiles(plan):
    """Yield (K, g_nodes_in_tile, slot_col_offset, node_col_offset) splits."""
    tiles = []
    off_s = 0
    off_n = 0
    for K, _, m, _ in plan:
        q = m // P
        gmax = max(1, F_TILE // K)
        i = 0
        while i < q:
            g = min(gmax, q - i)
            tiles.append((K, g, off_s + i * K, off_n + i))
            i += g
        off_s += q * K
        off_n += q
    return tiles


def _class_merges(plan):
    """Per-class (node_col_offset, near_col_offset, w) for near merges."""
    merges = []
    off_n = 0
    off_q = 0
    for K, _, m, w in plan:
        if w > 0:
            merges.append((off_n, off_q, w))
        off_n += m // P
        off_q += w
    return merges


def _raw_scalar_act(nc, out, in_, func, bias=0.0, scale=1.0):
    """InstActivation without the python wrapper's Reciprocal ban and without
    the const-AP bias conversion (immediates work for these funcs here)."""
    inputs = [nc.scalar.lower_ap(in_)]
    for arg in (bias, scale, 0.0):
        inputs.append(mybir.ImmediateValue(dtype=mybir.dt.float32, value=arg))
    return nc.scalar.add_instruction(
        mybir.InstActivation(
            name=nc.get_next_instruction_name(),
            func=func,
            ins=inputs,
            outs=[nc.scalar.lower_ap(out)],
        )
    )


def _build_nc(plan, NN, S, Wq, KN, DQ):
    """SPMD bass program: bf16 far pipeline + small exact fp32 near section."""
    fp32 = mybir.dt.float32
    bf16 = mybir.dt.bfloat16
    nc = bass.Bass("TRN2", target_bir_lowering=False)
    W = S // P
    Q = NN // P
    WN = max(Wq, 1) * KN

    cols = [
        nc.dram_tensor(f"col{i}", [P, W], bf16, kind="ExternalInput")
        for i in range(7)
    ]
    nodb = nc.dram_tensor("nodb", [P, 7 * Q], bf16, kind="ExternalInput")
    cntT = nc.dram_tensor("cnt", [P, Q], fp32, kind="ExternalInput")
    nearA = nc.dram_tensor("nearA", [P, 7 * WN], fp32, kind="ExternalInput")
    nearB = nc.dram_tensor("nearB", [P, 7 * WN], fp32, kind="ExternalInput")
    dlp = nc.dram_tensor("dlp", [P, 4 * DQ], fp32, kind="ExternalInput")
    dlt = nc.dram_tensor("dlt", [P, 4 * DQ], fp32, kind="ExternalInput")
    out = nc.dram_tensor("out", [P, 8], fp32, kind="ExternalOutput")

    AF = mybir.ActivationFunctionType
    OP = mybir.AluOpType

    with tile.TileContext(nc) as tc:
        with (
            tc.tile_pool(name="resident", bufs=1) as res_pool,
            tc.tile_pool(name="colp", bufs=2) as col_pool,
            tc.tile_pool(name="exp", bufs=2) as exp_pool,
            tc.tile_pool(name="tmp", bufs=2) as tmp_pool,
            tc.tile_pool(name="tmp1", bufs=1) as tmp1_pool,
        ):
            nodbt = res_pool.tile([P, 7 * Q], bf16)
            nc.sync.dma_start(nodbt[:], nodb.ap()[:])
            cntt = res_pool.tile([P, Q], fp32)
            nc.sync.dma_start(cntt[:], cntT.ap()[:])
            acc = res_pool.tile([P, 7 * Q], fp32)  # vg, lu, lv, lw, px, py, pz

            # ---- data loss (small, fp32) ----
            dlpt = res_pool.tile([P, 4 * DQ], fp32)
            nc.sync.dma_start(dlpt[:], dlp.ap()[:])
            dltt = res_pool.tile([P, 4 * DQ], fp32)
            nc.sync.dma_start(dltt[:], dlt.ap()[:])
            dld = res_pool.tile([P, 4 * DQ], fp32)
            nc.vector.tensor_sub(dld[:], dlpt[:], dltt[:])
            acc_vel = res_pool.tile([P, 1], fp32)
            acc_pres = res_pool.tile([P, 1], fp32)
            nc.scalar.activation(
                dld[:, 0 : 3 * DQ], dld[:, 0 : 3 * DQ], AF.Square,
                accum_out=acc_vel[:],
            )
            nc.scalar.activation(
                dld[:, 3 * DQ : 4 * DQ], dld[:, 3 * DQ : 4 * DQ], AF.Square,
                accum_out=acc_pres[:],
            )

            # ---- far loop (all bf16) ----
            acc4 = acc[:].rearrange("p (i q) -> p i q", i=7, q=Q)
            for K, g, off_s, off_n in _class_tiles(plan):
                F = g * K
                cvt = col_pool.tile([P, 7 * F], bf16, tag="cvt", name="cvt")
                for i in range(7):
                    nc.sync.dma_start(
                        cvt[:, i * F : (i + 1) * F],
                        cols[i].ap()[:, off_s : off_s + F],
                    )

                # node expand, all 7 planes in one ScalarE op
                evt = exp_pool.tile([P, 7 * F], bf16, tag="evt", name="evt")
                a7 = nodbt[:].rearrange("p (i q) -> p i q", i=7, q=Q)
                nc.scalar.copy(
                    evt[:].rearrange("p (i g k) -> p i g k", i=7, g=g, k=K),
                    a7[:, :, off_n : off_n + g]
                    .unsqueeze(-1)
                    .broadcast_to([P, 7, g, K]),
                )

                # all 7 diffs in one bf16 sub (2x)
                dft = tmp1_pool.tile([P, 7 * F], bf16, tag="dft", name="dft")
                nc.vector.tensor_sub(dft[:], cvt[:], evt[:])

                # r2 = dx^2+dy^2+dz^2 in bf16
                sqt = tmp_pool.tile([P, 3 * F], bf16, tag="sqt", name="sqt")
                nc.vector.tensor_mul(sqt[:], dft[:, 0 : 3 * F], dft[:, 0 : 3 * F])
                nc.vector.tensor_add(sqt[:, 0:F], sqt[:, 0:F], sqt[:, F : 2 * F])
                nc.vector.tensor_add(
                    sqt[:, 0:F], sqt[:, 0:F], sqt[:, 2 * F : 3 * F]
                )
                r2 = sqt[:, 0:F]

                # far edges have r2 >= tau >> eps, so 1/(sqrt(r2)+eps) ~=
                # rsqrt(r2) and 1/(r2+eps) ~= rsqrt(r2)^2 well inside bf16
                # noise. One LUT func -> no ACT table thrash.
                d1 = tmp_pool.tile([P, F], bf16, tag="d1")
                _raw_scalar_act(nc, d1[:], r2, AF.Rsqrt, bias=EPS)
                d2 = tmp_pool.tile([P, F], bf16, tag="d2")
                nc.vector.tensor_mul(d2[:], d1[:], d1[:])

                vals = tmp1_pool.tile([P, 7 * F], bf16, tag="vals", name="vals")
                # g = sum duvw*dxyz ; velgrad = g*d1 -> vals plane 0
                prt = tmp1_pool.tile([P, 3 * F], bf16, tag="prt", name="prt")
                nc.vector.tensor_mul(
                    prt[:], dft[:, 3 * F : 6 * F], dft[:, 0 : 3 * F]
                )
                nc.vector.tensor_add(prt[:, 0:F], prt[:, 0:F], prt[:, F : 2 * F])
                nc.vector.tensor_add(
                    prt[:, 0:F], prt[:, 0:F], prt[:, 2 * F : 3 * F]
                )
                nc.vector.tensor_mul(vals[:, 0:F], prt[:, 0:F], d1[:])

                # cp = dq*d1*d1 (in-place in dft plane 6)
                cpq = dft[:, 6 * F : 7 * F]
                nc.vector.tensor_mul(cpq, cpq, d1[:])
                nc.vector.tensor_mul(cpq, cpq, d1[:])

                # pg = cp * dxyz -> vals planes 4:7 (cp broadcast over 3)
                cp3 = bass.AP(dft[:].tensor, dft[:].offset + 6 * F,
                              [dft[:].ap[0], [0, 3], [1, F]])
                nc.vector.tensor_mul(vals[:, 4 * F : 7 * F].rearrange(
                    "p (i f) -> p i f", i=3, f=F), cp3,
                    dft[:, 0 : 3 * F].rearrange("p (i f) -> p i f", i=3, f=F))
                # lap = duvw * d2 -> vals planes 1:4 (d2 broadcast over 3)
                d23 = bass.AP(d2[:].tensor, d2[:].offset,
                              [d2[:].ap[0], [0, 3], [1, F]])
                nc.vector.tensor_mul(vals[:, 1 * F : 4 * F].rearrange(
                    "p (i f) -> p i f", i=3, f=F),
                    dft[:, 3 * F : 6 * F].rearrange("p (i f) -> p i f", i=3, f=F),
                    d23)

                # segment sums: aligned halving tree over K down to 2
                v4 = vals[:].rearrange("p (i g k) -> p i g k", i=7, g=g, k=K)
                k = K
                while k > 2:
                    if k % 4 == 2:
                        nc.vector.tensor_add(
                            v4[:, :, :, 0:2], v4[:, :, :, 0:2],
                            v4[:, :, :, k - 2 : k],
                        )
                        k -= 2
                    else:
                        h = k // 2
                        nc.vector.tensor_add(
                            v4[:, :, :, 0:h], v4[:, :, :, 0:h],
                            v4[:, :, :, h:k],
                        )
                        k = h
                nc.vector.tensor_reduce(
                    acc4[:, :, off_n : off_n + g],
                    v4[:, :, :, 0:k],
                    mybir.AxisListType.X,
                    OP.add,
                )

            # ---- near section (exact fp32, tiny) ----
            if Wq > 0:
                nat = col_pool.tile([P, 7 * WN], fp32, tag="cvt", name="nat")
                nc.sync.dma_start(nat[:], nearA.ap()[:])
                nbt = exp_pool.tile([P, 7 * WN], fp32, tag="evt", name="nbt")
                nc.sync.dma_start(nbt[:], nearB.ap()[:])
                nd = tmp1_pool.tile([P, 7 * WN], fp32, tag="dft", name="nd")
                nc.vector.tensor_sub(nd[:], nat[:], nbt[:])
                nsq = tmp_pool.tile([P, 3 * WN], fp32, tag="sqt", name="nsq")
                nc.scalar.activation(nsq[:], nd[:, 0 : 3 * WN], AF.Square)
                nc.vector.tensor_add(
                    nsq[:, 0:WN], nsq[:, 0:WN], nsq[:, WN : 2 * WN]
                )
                nc.vector.tensor_add(
                    nsq[:, 0:WN], nsq[:, 0:WN], nsq[:, 2 * WN : 3 * WN]
                )
                r2n = nsq[:, 0:WN]
                ssn = tmp_pool.tile([P, WN], fp32, tag="ss", name="ssn")
                nc.scalar.activation(ssn[:], r2n, AF.Sqrt)
                d1n = tmp_pool.tile([P, WN], fp32, tag="d1", name="d1n")
                _raw_scalar_act(nc, d1n[:], ssn[:], AF.Reciprocal, bias=EPS)
                d2n = tmp_pool.tile([P, WN], fp32, tag="d2", name="d2n")
                _raw_scalar_act(nc, d2n[:], r2n, AF.Reciprocal, bias=EPS)

                nvals = tmp1_pool.tile([P, 7 * WN], fp32, tag="vals", name="nvals")
                prn = tmp1_pool.tile([P, 3 * WN], fp32, tag="prt", name="prn")
                nc.vector.tensor_mul(
                    prn[:], nd[:, 3 * WN : 6 * WN], nd[:, 0 : 3 * WN]
                )
                nc.vector.tensor_add(
                    prn[:, 0:WN], prn[:, 0:WN], prn[:, WN : 2 * WN]
                )
                nc.vector.tensor_add(
                    prn[:, 0:WN], prn[:, 0:WN], prn[:, 2 * WN : 3 * WN]
                )
                nc.vector.tensor_mul(nvals[:, 0:WN], prn[:, 0:WN], d1n[:])
                cpn = nd[:, 6 * WN : 7 * WN]
                nc.vector.tensor_mul(cpn, cpn, d1n[:])
                nc.vector.tensor_mul(cpn, cpn, d1n[:])
                cp3n = bass.AP(nd[:].tensor, nd[:].offset + 6 * WN,
                               [nd[:].ap[0], [0, 3], [1, WN]])
                nc.vector.tensor_mul(
                    nvals[:, 4 * WN : 7 * WN].rearrange(
                        "p (i f) -> p i f", i=3, f=WN),
                    cp3n,
                    nd[:, 0 : 3 * WN].rearrange("p (i f) -> p i f", i=3, f=WN))
                d23n = bass.AP(d2n[:].tensor, d2n[:].offset,
                               [d2n[:].ap[0], [0, 3], [1, WN]])
                nc.vector.tensor_mul(
                    nvals[:, 1 * WN : 4 * WN].rearrange(
                        "p (i f) -> p i f", i=3, f=WN),
                    nd[:, 3 * WN : 6 * WN].rearrange("p (i f) -> p i f", i=3, f=WN),
                    d23n)

                accN = res_pool.tile([P, 7 * Wq], fp32)
                nv4 = nvals[:].rearrange(
                    "p (i w k) -> p i w k", i=7, w=Wq, k=KN
                )
                k = KN
                while k > 2:
                    h = k // 2
                    nc.vector.tensor_add(
                        nv4[:, :, :, 0:h], nv4[:, :, :, 0:h], nv4[:, :, :, h:k]
                    )
                    k = h
                accN4 = accN[:].rearrange("p (i w) -> p i w", i=7, w=Wq)
                nc.vector.tensor_reduce(
                    accN4, nv4[:, :, :, 0:k], mybir.AxisListType.X, OP.add
                )
                # merge near partial sums into acc (aligned contiguous ranges)
                for off_n, off_q, w in _class_merges(plan):
                    nc.vector.tensor_add(
                        acc4[:, :, off_n : off_n + w],
                        acc4[:, :, off_n : off_n + w],
                        accN4[:, :, off_q : off_q + w],
                    )

            # ---- finish (fp32, small) ----
            icnt = res_pool.tile([P, Q], fp32)
            _raw_scalar_act(nc, icnt[:], cntt[:], AF.Reciprocal)
            div = res_pool.tile([P, Q], fp32)
            nc.vector.tensor_mul(div[:], acc[:, 0:Q], icnt[:])
            acc_div2 = res_pool.tile([P, 1], fp32)
            nc.scalar.activation(div[:], div[:], AF.Square, accum_out=acc_div2[:])
            acc_m = [
                res_pool.tile([P, 1], fp32, tag=f"am{i}", name=f"am{i}")
                for i in range(3)
            ]
            for i in range(3):
                r = res_pool.tile([P, Q], fp32, tag="rfin")
                nc.vector.scalar_tensor_tensor(
                    r[:],
                    acc[:, (1 + i) * Q : (2 + i) * Q],
                    1.0 / REYNOLDS,
                    acc[:, (4 + i) * Q : (5 + i) * Q],
                    OP.mult,
                    OP.add,
                )
                nc.vector.tensor_mul(r[:], r[:], icnt[:])
                nc.scalar.activation(r[:], r[:], AF.Square, accum_out=acc_m[i][:])

            outt = res_pool.tile([P, 8], fp32)
            nc.vector.memset(outt[:], 0.0)
            nc.vector.tensor_copy(outt[:, 0:1], acc_vel[:])
            nc.vector.tensor_copy(outt[:, 1:2], acc_pres[:])
            nc.vector.tensor_copy(outt[:, 2:3], acc_div2[:])
            nc.vector.tensor_copy(outt[:, 3:4], acc_m[0][:])
            nc.vector.tensor_copy(outt[:, 4:5], acc_m[1][:])
            nc.vector.tensor_copy(outt[:, 5:6], acc_m[2][:])
            nc.sync.dma_start(out.ap()[:], outt[:])

    return nc


# ---------------------------------------------------------------- entry

_CACHE = {}


def _get_nc(key, plan, NN, S, Wq, KN, DQ):
    if key not in _CACHE:
        _CACHE[key] = _build_nc(plan, NN, S, Wq, KN, DQ)
    return _CACHE[key]


LAST_RESULT = None  # BassKernelResults of the most recent run (for profiling)


def kernel(pred, target, edge_index, pos, _trace_dir=None):
    global LAST_RESULT
    pred = np.asarray(pred)
    target = np.asarray(target)
    pos = np.asarray(pos).astype(np.float32)
    row = np.asarray(edge_index[0]).astype(np.int64)
    col = np.asarray(edge_index[1]).astype(np.int64)
    n = pred.shape[0]

    plan, deg, dnear, dfar, offs, col_s2, KN = _build_plan(row, col, pos, n)
    nodedata = np.concatenate(
        [pos, pred.astype(np.float32)], axis=1
    )
    cores, NN, S, Wq = _build_streams(
        plan, deg, dnear, dfar, offs, col_s2, KN, nodedata
    )
    Qn = NN // P
    WN = max(Wq, 1) * KN

    # data-loss slices: split all n nodes across cores, pad to mult of 128
    per = -(-n // N_CORES)
    DQ = -(-per // P)
    predf = pred.astype(np.float32)
    targf = target.astype(np.float32)

    in_maps = []
    for c in range(N_CORES):
        lo, hi = c * per, min((c + 1) * per, n)
        ps = np.zeros((P * DQ, 4), np.float32)
        ts = np.zeros((P * DQ, 4), np.float32)
        ps[: hi - lo] = predf[lo:hi]
        ts[: hi - lo] = targf[lo:hi]
        # [P, 4*DQ] with plane-major columns: plane i at cols [i*DQ, (i+1)*DQ)
        dlp = ps.reshape(P, DQ, 4).transpose(0, 2, 1).reshape(P, 4 * DQ)
        dlt = ts.reshape(P, DQ, 4).transpose(0, 2, 1).reshape(P, 4 * DQ)
        cc = cores[c]
        m = dict(
            cnt=np.ascontiguousarray(cc["cnt"]),
            nodb=np.ascontiguousarray(
                cc["nodb"].transpose(1, 0, 2).reshape(P, 7 * Qn)
            ),
            nearA=np.ascontiguousarray(
                cc["nearA"].transpose(1, 0, 2).reshape(P, 7 * (Wq * KN))
            ) if Wq > 0 else np.zeros((P, 7 * WN), np.float32),
            nearB=np.ascontiguousarray(
                cc["nearB"].transpose(1, 0, 2).reshape(P, 7 * (Wq * KN))
            ) if Wq > 0 else np.zeros((P, 7 * WN), np.float32),
            dlp=np.ascontiguousarray(dlp),
            dlt=np.ascontiguousarray(dlt),
        )
        for i in range(7):
            m[f"col{i}"] = np.ascontiguousarray(cc["colp"][i])
        in_maps.append(m)

    key = (tuple((K, mm, w) for K, _, mm, w in plan), NN, S, Wq, KN, DQ)
    plan_geom = [(K, None, mm, w) for K, _, mm, w in plan]
    nc = _get_nc(key, plan_geom, NN, S, Wq, KN, DQ)

    if _trace_dir is not None:
        _install_ntff_hook()
        res = run_bass_kernel_spmd(
            nc, in_maps, core_ids=list(range(N_CORES)), trace=True,
            tmpdir=_trace_dir,
        )
    else:
        res = run_bass_kernel_spmd(nc, in_maps, core_ids=list(range(N_CORES)))
    LAST_RESULT = res

    tot = np.zeros(8, np.float64)
    for c in range(N_CORES):
        tot += res.results[c]["out"].astype(np.float64).sum(axis=0)
    s_vel, s_pres, s_div2, am0, am1, am2 = (
        tot[0], tot[1], tot[2], tot[3], tot[4], tot[5]
    )
    loss = (
        s_vel / (3 * n)
        + s_pres / n
        + LAMBDA_CONT * s_div2 / n
        + LAMBDA_MOM * (am0 + am1 + am2) / (3 * n)
    )
    return np.float32(loss)


# revision 38
# speedup vs baseline: 1.2447x; 1.0666x over previous
"""PhysicsInformedLoss on 8 Trainium2 NeuronCores.

Sharding strategy (degree-class padded CSR):
- Edges are grouped by destination node `row` (the scatter target of every
  segment-mean in the reference). Nodes with deg>0 are binned into degree
  classes K (multiples of 4, small classes merged); each node gets exactly K
  contiguous "slots" (its edges + self-pads, pads contribute exactly 0).
- Nodes of each class are split evenly across the 8 cores (identical padded
  per-core counts -> one SPMD program). Per core, node i of a class maps to
  partition p = i // q (q nodes/partition), so every per-node segment sum is
  a static strided reduction along the free dimension.
- The host gathers the 7 col-side planes (pos xyz, vel uvw, p) in slot order
  (this is the "shard the edges" data layout step); the row side is the
  per-node resident plane broadcast along K by a stride-0 access pattern.
- Device per core: stream col planes, compute all per-edge terms, strided
  per-node reduce, finish div/residual/squares; output per-partition partial
  sums. Host sums 8x128 partials and forms the scalar loss.
"""
import contextlib
import ctypes
import os
import sys
import tempfile
import types

import numpy as np

import concourse.bass as bass
import concourse.tile as tile
from concourse import mybir
from concourse.vector_clock import ScopedClock
from concourse.bass_utils import run_bass_kernel_spmd

N_CORES = 8
P = 128
EPS = 1e-8
REYNOLDS = 1000000.0
LAMBDA_CONT = 0.1
LAMBDA_MOM = 0.01
F_TILE = 1792  # target per-partition columns per tile

# ---------------------------------------------------------------- tile patch
# walrus in this environment allows only ONE sync-wait per instruction, but
# Tile's scheduler can emit several. Split surplus waits onto engine NOPs
# inserted right before the offending instruction.
_MAX_WAITS = 1


def _split_multi_waits(nc, handles):
    work = []
    for fn in nc.m.functions:
        for bb in fn.blocks:
            items = []
            for inst in bb.instructions:
                si = inst.sync_info
                waits = list(si.on_wait) if si and si.on_wait else []
                if len(waits) > _MAX_WAITS:
                    keep = len(waits) - _MAX_WAITS
                    extra = waits[:keep]
                    si.on_wait = waits[keep:]
                    chunks = [
                        extra[i : i + _MAX_WAITS]
                        for i in range(0, len(extra), _MAX_WAITS)
                    ]
                    items.append((inst.name, inst.engine, chunks))
            if items:
                work.append((bb, items))
    if not work:
        return
    created = {}
    placements = {}
    for bb, items in work:
        plc = {}
        for inst_name, engine, chunks in items:
            nops = []
            for chunk in chunks:
                for w in chunk:
                    h = handles.get(w.ant_name)
                    assert h is not None, f"no sem handle for {w.ant_name}"
                    ni = nc.engines[engine].wait_ge(h, w.wait_value)
                    created[ni.ins.name] = None
                    nops.append(ni.ins)
            plc[inst_name] = nops
        placements[id(bb)] = plc
    for fn in nc.m.functions:
        for bb in fn.blocks:
            plc = placements.get(id(bb), {})
            newlist = []
            for inst in bb.instructions:
                if inst.name in created:
                    continue
                if inst.name in plc:
                    newlist.extend(plc[inst.name])
                newlist.append(inst)
            bb.instructions = newlist


def _patched_drain_and_barrier(self, tick_clock, wait_clock):
    drain_inst = self.nc.sync.drain()
    wait_clock.add_sem_waits(
        drain_inst.ins, ScopedClock({None: tick_clock.global_clock})
    )
    handles = {h.name: h for h in self.sems.allocated().values()}
    _split_multi_waits(self.nc, handles)
    self.nc.all_engine_barrier()
    popped = self.nc._tile_sem_poison_stack.pop()
    assert popped is self._sem_poison
    self.nc.clear_and_free_semaphores(list(self.sems.allocated().values()))
    self.nc.all_engine_barrier()


tile.TileContext._drain_and_barrier = _patched_drain_and_barrier

# ------------------------------------------------------------- ntff hook
# The env's antenv package lacks axon_hooks; recreate the NTFF profile hook
# via ctypes so run_bass_kernel_spmd(trace=True) works (test/profiling only).
_AXON_SO = "/opt/axon/libaxon_pjrt.so"


def _install_ntff_hook():
    if "antenv.axon_hooks" in sys.modules:
        return
    try:
        lib = ctypes.CDLL(_AXON_SO)
        lib.axon_start_nrt_profile.argtypes = [
            ctypes.POINTER(ctypes.c_int64),
            ctypes.c_size_t,
        ]
        lib.axon_start_nrt_profile.restype = ctypes.c_int64
        lib.axon_stop_nrt_profile.argtypes = [ctypes.c_char_p]
        lib.axon_stop_nrt_profile.restype = ctypes.c_int64
    except Exception:
        return

    @contextlib.contextmanager
    def _hook(output_dir, device_ids):
        import jax

        jax.devices()
        if device_ids:
            ids = (ctypes.c_int64 * len(device_ids))(*device_ids)
            rc = lib.axon_start_nrt_profile(ids, len(device_ids))
        else:
            rc = lib.axon_start_nrt_profile(None, 0)
        if rc != 0:
            raise RuntimeError(f"axon_start_nrt_profile rc={rc}")
        try:
            yield
        finally:
            n = lib.axon_stop_nrt_profile(str(output_dir).encode())
            print(f"profile: {n} file(s) written to {output_dir}", file=sys.stderr)

    mod = types.ModuleType("antenv.axon_hooks")
    mod.get_axon_ntff_profile_hook = lambda: _hook
    mod.set_axon_ntff_profile_hook = lambda h: None
    sys.modules["antenv.axon_hooks"] = mod


# ---------------------------------------------------------------- host prep


def _build_plan(row, col, pos, n, tau=1e-2, knear=8, min_class_slots=128 * 1024):
    """Split edges into far (bf16-safe, dist^2 >= tau) and near (exact fp32
    section, <= knear per node; overflow demoted to far). Returns plan with
    per-class far geometry + near geometry."""
    deg = np.bincount(row, minlength=n).astype(np.int64)
    order = np.argsort(row, kind="stable")
    row_s = row[order]
    col_s = col[order]
    pd = pos[col_s].astype(np.float32) - pos[row_s].astype(np.float32)
    r2e = (pd * pd).sum(1)
    near_e = r2e < np.float32(tau)
    # within each node's run, put near edges first (stable)
    order2 = np.argsort(row_s * 2 + (~near_e), kind="stable")
    col_s2 = col_s[order2]
    offs = np.zeros(n + 1, dtype=np.int64)
    np.cumsum(deg, out=offs[1:])
    dnear = np.bincount(row_s[near_e], minlength=n)
    dnear = np.minimum(dnear, knear).astype(np.int64)
    dfar = deg - dnear
    affected = dnear > 0

    active = deg > 0
    kraw = ((np.maximum(dfar, 1) + 3) // 4) * 4
    uniq = np.unique(kraw[active])
    groups = []
    pend = []
    pend_slots = 0
    for K in uniq:
        ids = np.nonzero(active & (kraw == K))[0]
        pend.append(ids)
        pend_slots += ids.size * int(K)
        if pend_slots >= min_class_slots or K == uniq[-1]:
            allids = np.concatenate(pend)
            groups.append((int(K), allids))
            pend = []
            pend_slots = 0

    plan = []
    for K, ids in groups:
        aff = affected[ids]
        # affected first (stable), then round-robin core split
        ids = ids[np.argsort(~aff, kind="stable")]
        per_core = -(-ids.size // N_CORES)
        m = -(-per_core // P) * P  # per-core nodes, padded to mult of 128
        a_max = 0
        for c in range(N_CORES):
            a_max = max(a_max, int(affected[ids[c::N_CORES]].sum()))
        w = -(-a_max // P)  # near cols per class (common)
        plan.append((K, ids, m, w))
    return plan, deg, dnear, dfar, offs, col_s2, knear


def _build_streams(plan, deg, dnear, dfar, offs, col_s2, knear, nodedata):
    """nodedata: [n,7] f32. Builds per-core far bf16 streams (interleaved
    node->partition mapping) + near fp32 section. Returns cores, NN, S, Wq."""
    import ml_dtypes

    bf = ml_dtypes.bfloat16
    S = sum(m * K for K, _, m, _ in plan)
    NN = sum(m for _, _, m, _ in plan)
    Wq = sum(w for _, _, _, w in plan)
    cores = []
    for c in range(N_CORES):
        colp = np.zeros((7, P, S // P), bf)
        nodb = np.zeros((7, P, NN // P), bf)
        cnt = np.ones((P, NN // P), np.float32)
        nearA = np.zeros((7, P, Wq * knear), np.float32)
        nearB = np.zeros((7, P, Wq * knear), np.float32)
        off_slots = 0
        off_nodes = 0
        off_q = 0
        for K, ids, m, w in plan:
            q = m // P
            ids_c = ids[c::N_CORES]
            k_real = ids_c.size
            # ---- far stream ----
            vals = np.zeros((m, K, 7), np.float32)
            nodevals = np.zeros((m, 7), np.float32)
            cv = np.ones(m, np.float32)
            if k_real > 0:
                colmat = np.empty((k_real, K), np.int64)
                colmat[:] = ids_c[:, None]
                df = dfar[ids_c]
                oo = offs[ids_c] + dnear[ids_c]  # far edges after near ones
                ar = np.arange(K)[None, :]
                valid = ar < df[:, None]
                src_idx = (oo[:, None] + ar)[valid]
                colmat[valid] = col_s2[src_idx]
                vals[:k_real] = nodedata[colmat]
                nodevals[:k_real] = nodedata[ids_c]
                cv[:k_real] = np.maximum(deg[ids_c], 1).astype(np.float32)
            wv = vals.reshape(q, P, K, 7).transpose(3, 1, 0, 2).reshape(7, P, q * K)
            colp[:, :, off_slots : off_slots + q * K] = wv.astype(bf)
            nodb[:, :, off_nodes : off_nodes + q] = (
                nodevals.reshape(q, P, 7).transpose(2, 1, 0).astype(bf)
            )
            cnt[:, off_nodes : off_nodes + q] = cv.reshape(q, P).T
            # ---- near section (affected nodes = first a of ids_c) ----
            if w > 0:
                mw = w * P
                a = int((dnear[ids_c] > 0).sum()) if k_real > 0 else 0
                nvals = np.zeros((mw, knear, 7), np.float32)
                nnode = np.zeros((mw, knear, 7), np.float32)
                if a > 0:
                    aid = ids_c[:a]
                    colm = np.empty((a, knear), np.int64)
                    colm[:] = aid[:, None]
                    dn = dnear[aid]
                    oo = offs[aid]
                    ar = np.arange(knear)[None, :]
                    valid = ar < dn[:, None]
                    src_idx = (oo[:, None] + ar)[valid]
                    colm[valid] = col_s2[src_idx]
                    nvals[:a] = nodedata[colm]
                    nnode[:a] = nodedata[aid][:, None, :]
                wa = nvals.reshape(w, P, knear, 7).transpose(3, 1, 0, 2)
                wb = nnode.reshape(w, P, knear, 7).transpose(3, 1, 0, 2)
                nearA[:, :, off_q * knear : (off_q + w) * knear] = wa.reshape(
                    7, P, w * knear
                )
                nearB[:, :, off_q * knear : (off_q + w) * knear] = wb.reshape(
                    7, P, w * knear
                )
            off_slots += q * K
            off_nodes += q
            off_q += w
        cores.append(dict(colp=colp, nodb=nodb, cnt=cnt, nearA=nearA, nearB=nearB))
    return cores, NN, S, Wq


# ---------------------------------------------------------------- bass build


def _class_tiles(plan):
    """Yield (K, g_nodes_in_tile, slot_col_offset, node_col_offset) splits."""
    tiles = []
    off_s = 0
    off_n = 0
    for K, _, m, _ in plan:
        q = m // P
        gmax = max(1, F_TILE // K)
        i = 0
        while i < q:
            g = min(gmax, q - i)
            tiles.append((K, g, off_s + i * K, off_n + i))
            i += g
        off_s += q * K
        off_n += q
    return tiles


def _class_merges(plan):
    """Per-class (node_col_offset, near_col_offset, w) for near merges."""
    merges = []
    off_n = 0
    off_q = 0
    for K, _, m, w in plan:
        if w > 0:
            merges.append((off_n, off_q, w))
        off_n += m // P
        off_q += w
    return merges


def _raw_scalar_act(nc, out, in_, func, bias=0.0, scale=1.0):
    """InstActivation without the python wrapper's Reciprocal ban and without
    the const-AP bias conversion (immediates work for these funcs here)."""
    inputs = [nc.scalar.lower_ap(in_)]
    for arg in (bias, scale, 0.0):
        inputs.append(mybir.ImmediateValue(dtype=mybir.dt.float32, value=arg))
    return nc.scalar.add_instruction(
        mybir.InstActivation(
            name=nc.get_next_instruction_name(),
            func=func,
            ins=inputs,
            outs=[nc.scalar.lower_ap(out)],
        )
    )


def _build_nc(plan, NN, S, Wq, KN, DQ):
    """SPMD bass program: bf16 far pipeline + small exact fp32 near section."""
    fp32 = mybir.dt.float32
    bf16 = mybir.dt.bfloat16
    nc = bass.Bass("TRN2", target_bir_lowering=False)
    W = S // P
    Q = NN // P
    WN = max(Wq, 1) * KN

    colT = nc.dram_tensor("colT", [7 * P, W], bf16, kind="ExternalInput")
    nodb = nc.dram_tensor("nodb", [P, 7 * Q], bf16, kind="ExternalInput")
    cntT = nc.dram_tensor("cnt", [P, Q], fp32, kind="ExternalInput")
    nearA = nc.dram_tensor("nearA", [P, 7 * WN], fp32, kind="ExternalInput")
    nearB = nc.dram_tensor("nearB", [P, 7 * WN], fp32, kind="ExternalInput")
    dlp = nc.dram_tensor("dlp", [P, 4 * DQ], fp32, kind="ExternalInput")
    dlt = nc.dram_tensor("dlt", [P, 4 * DQ], fp32, kind="ExternalInput")
    out = nc.dram_tensor("out", [P, 8], fp32, kind="ExternalOutput")

    AF = mybir.ActivationFunctionType
    OP = mybir.AluOpType

    with tile.TileContext(nc) as tc:
        with (
            tc.tile_pool(name="resident", bufs=1) as res_pool,
            tc.tile_pool(name="colp", bufs=2) as col_pool,
            tc.tile_pool(name="exp", bufs=2) as exp_pool,
            tc.tile_pool(name="tmp", bufs=2) as tmp_pool,
            tc.tile_pool(name="tmp1", bufs=2) as tmp1_pool,
        ):
            nodbt = res_pool.tile([P, 7 * Q], bf16)
            nc.sync.dma_start(nodbt[:], nodb.ap()[:])
            cntt = res_pool.tile([P, Q], fp32)
            nc.sync.dma_start(cntt[:], cntT.ap()[:])
            acc = res_pool.tile([P, 7 * Q], fp32)  # vg, lu, lv, lw, px, py, pz

            # ---- data loss (small, fp32) ----
            dlpt = res_pool.tile([P, 4 * DQ], fp32)
            nc.sync.dma_start(dlpt[:], dlp.ap()[:])
            dltt = res_pool.tile([P, 4 * DQ], fp32)
            nc.sync.dma_start(dltt[:], dlt.ap()[:])
            dld = res_pool.tile([P, 4 * DQ], fp32)
            nc.vector.tensor_sub(dld[:], dlpt[:], dltt[:])
            acc_vel = res_pool.tile([P, 1], fp32)
            acc_pres = res_pool.tile([P, 1], fp32)
            nc.scalar.activation(
                dld[:, 0 : 3 * DQ], dld[:, 0 : 3 * DQ], AF.Square,
                accum_out=acc_vel[:],
            )
            nc.scalar.activation(
                dld[:, 3 * DQ : 4 * DQ], dld[:, 3 * DQ : 4 * DQ], AF.Square,
                accum_out=acc_pres[:],
            )

            # ---- far loop (all bf16) ----
            acc4 = acc[:].rearrange("p (i q) -> p i q", i=7, q=Q)
            for K, g, off_s, off_n in _class_tiles(plan):
                F = g * K
                # negated node expand written into cvt by ScalarE, then the
                # column load lands on top with a CCE add: cvt = col - node.
                cvt = col_pool.tile([P, 7 * F], bf16, tag="cvt", name="cvt")
                a7 = nodbt[:].rearrange("p (i q) -> p i q", i=7, q=Q)
                nc.scalar.mul(
                    cvt[:].rearrange("p (i g k) -> p i g k", i=7, g=g, k=K),
                    a7[:, :, off_n : off_n + g]
                    .unsqueeze(-1)
                    .broadcast_to([P, 7, g, K]),
                    -1.0,
                )
                nc.gpsimd.dma_start(
                    cvt[:].rearrange("p (i f) -> p i f", i=7, f=F),
                    bass.AP(colT, off_s, [[W, P], [P * W, 7], [1, F]]),
                    accum_op=OP.add,
                )
                dft = cvt

                # r2 = dx^2+dy^2+dz^2 in bf16
                sqt = tmp_pool.tile([P, 3 * F], bf16, tag="sqt", name="sqt")
                nc.vector.tensor_mul(sqt[:], cvt[:, 0 : 3 * F], cvt[:, 0 : 3 * F])
                nc.vector.tensor_add(sqt[:, 0:F], sqt[:, 0:F], sqt[:, F : 2 * F])
                nc.vector.tensor_add(
                    sqt[:, 0:F], sqt[:, 0:F], sqt[:, 2 * F : 3 * F]
                )
                r2 = sqt[:, 0:F]

                # far edges have r2 >= tau >> eps, so 1/(sqrt(r2)+eps) ~=
                # rsqrt(r2) and 1/(r2+eps) ~= rsqrt(r2)^2 well inside bf16
                # noise. One LUT func -> no ACT table thrash.
                d1 = tmp_pool.tile([P, F], bf16, tag="d1")
                _raw_scalar_act(nc, d1[:], r2, AF.Rsqrt, bias=EPS)
                d2 = tmp_pool.tile([P, F], bf16, tag="d2")
                nc.vector.tensor_mul(d2[:], d1[:], d1[:])

                vals = tmp1_pool.tile([P, 7 * F], bf16, tag="vals", name="vals")
                # g = sum duvw*dxyz ; velgrad = g*d1 -> vals plane 0
                prt = tmp1_pool.tile([P, 3 * F], bf16, tag="prt", name="prt")
                nc.vector.tensor_mul(
                    prt[:], cvt[:, 3 * F : 6 * F], cvt[:, 0 : 3 * F]
                )
                nc.vector.tensor_add(prt[:, 0:F], prt[:, 0:F], prt[:, F : 2 * F])
                nc.vector.tensor_add(
                    prt[:, 0:F], prt[:, 0:F], prt[:, 2 * F : 3 * F]
                )
                nc.vector.tensor_mul(vals[:, 0:F], prt[:, 0:F], d1[:])

                # cp = dq*d1*d1 = dq*d2 (in-place in dft plane 6)
                cpq = cvt[:, 6 * F : 7 * F]
                nc.vector.tensor_mul(cpq, cpq, d2[:])

                # pg = cp * dxyz -> vals planes 4:7 (cp broadcast over 3)
                cp3 = bass.AP(cvt[:].tensor, cvt[:].offset + 6 * F,
                              [cvt[:].ap[0], [0, 3], [1, F]])
                nc.vector.tensor_mul(vals[:, 4 * F : 7 * F].rearrange(
                    "p (i f) -> p i f", i=3, f=F), cp3,
                    cvt[:, 0 : 3 * F].rearrange("p (i f) -> p i f", i=3, f=F))
                # lap = duvw * d2 -> vals planes 1:4 (d2 broadcast over 3)
                d23 = bass.AP(d2[:].tensor, d2[:].offset,
                              [d2[:].ap[0], [0, 3], [1, F]])
                nc.vector.tensor_mul(vals[:, 1 * F : 4 * F].rearrange(
                    "p (i f) -> p i f", i=3, f=F),
                    cvt[:, 3 * F : 6 * F].rearrange("p (i f) -> p i f", i=3, f=F),
                    d23)

                # segment sums: aligned halving tree over K down to 2
                v4 = vals[:].rearrange("p (i g k) -> p i g k", i=7, g=g, k=K)
                k = K
                while k > 2:
                    if k % 4 == 2:
                        nc.vector.tensor_add(
                            v4[:, :, :, 0:2], v4[:, :, :, 0:2],
                            v4[:, :, :, k - 2 : k],
                        )
                        k -= 2
                    else:
                        h = k // 2
                        nc.vector.tensor_add(
                            v4[:, :, :, 0:h], v4[:, :, :, 0:h],
                            v4[:, :, :, h:k],
                        )
                        k = h
                nc.vector.tensor_reduce(
                    acc4[:, :, off_n : off_n + g],
                    v4[:, :, :, 0:k],
                    mybir.AxisListType.X,
                    OP.add,
                )

            # ---- near section (exact fp32, tiny) ----
            if Wq > 0:
                nat = col_pool.tile([P, 7 * WN], fp32, tag="cvt", name="nat")
                nc.sync.dma_start(nat[:], nearA.ap()[:])
                nbt = exp_pool.tile([P, 7 * WN], fp32, tag="evt", name="nbt")
                nc.sync.dma_start(nbt[:], nearB.ap()[:])
                nd = tmp1_pool.tile([P, 7 * WN], fp32, tag="dft", name="nd")
                nc.vector.tensor_sub(nd[:], nat[:], nbt[:])
                nsq = tmp_pool.tile([P, 3 * WN], fp32, tag="sqt", name="nsq")
                nc.scalar.activation(nsq[:], nd[:, 0 : 3 * WN], AF.Square)
                nc.vector.tensor_add(
                    nsq[:, 0:WN], nsq[:, 0:WN], nsq[:, WN : 2 * WN]
                )
                nc.vector.tensor_add(
                    nsq[:, 0:WN], nsq[:, 0:WN], nsq[:, 2 * WN : 3 * WN]
                )
                r2n = nsq[:, 0:WN]
                ssn = tmp_pool.tile([P, WN], fp32, tag="ss", name="ssn")
                nc.scalar.activation(ssn[:], r2n, AF.Sqrt)
                d1n = tmp_pool.tile([P, WN], fp32, tag="d1", name="d1n")
                _raw_scalar_act(nc, d1n[:], ssn[:], AF.Reciprocal, bias=EPS)
                d2n = tmp_pool.tile([P, WN], fp32, tag="d2", name="d2n")
                _raw_scalar_act(nc, d2n[:], r2n, AF.Reciprocal, bias=EPS)

                nvals = tmp1_pool.tile([P, 7 * WN], fp32, tag="vals", name="nvals")
                prn = tmp1_pool.tile([P, 3 * WN], fp32, tag="prt", name="prn")
                nc.vector.tensor_mul(
                    prn[:], nd[:, 3 * WN : 6 * WN], nd[:, 0 : 3 * WN]
                )
                nc.vector.tensor_add(
                    prn[:, 0:WN], prn[:, 0:WN], prn[:, WN : 2 * WN]
                )
                nc.vector.tensor_add(
                    prn[:, 0:WN], prn[:, 0:WN], prn[:, 2 * WN : 3 * WN]
                )
                nc.vector.tensor_mul(nvals[:, 0:WN], prn[:, 0:WN], d1n[:])
                cpn = nd[:, 6 * WN : 7 * WN]
                nc.vector.tensor_mul(cpn, cpn, d1n[:])
                nc.vector.tensor_mul(cpn, cpn, d1n[:])
                cp3n = bass.AP(nd[:].tensor, nd[:].offset + 6 * WN,
                               [nd[:].ap[0], [0, 3], [1, WN]])
                nc.vector.tensor_mul(
                    nvals[:, 4 * WN : 7 * WN].rearrange(
                        "p (i f) -> p i f", i=3, f=WN),
                    cp3n,
                    nd[:, 0 : 3 * WN].rearrange("p (i f) -> p i f", i=3, f=WN))
                d23n = bass.AP(d2n[:].tensor, d2n[:].offset,
                               [d2n[:].ap[0], [0, 3], [1, WN]])
                nc.vector.tensor_mul(
                    nvals[:, 1 * WN : 4 * WN].rearrange(
                        "p (i f) -> p i f", i=3, f=WN),
                    nd[:, 3 * WN : 6 * WN].rearrange("p (i f) -> p i f", i=3, f=WN),
                    d23n)

                accN = res_pool.tile([P, 7 * Wq], fp32)
                nv4 = nvals[:].rearrange(
                    "p (i w k) -> p i w k", i=7, w=Wq, k=KN
                )
                k = KN
                while k > 2:
                    h = k // 2
                    nc.vector.tensor_add(
                        nv4[:, :, :, 0:h], nv4[:, :, :, 0:h], nv4[:, :, :, h:k]
                    )
                    k = h
                accN4 = accN[:].rearrange("p (i w) -> p i w", i=7, w=Wq)
                nc.vector.tensor_reduce(
                    accN4, nv4[:, :, :, 0:k], mybir.AxisListType.X, OP.add
                )
                # merge near partial sums into acc (aligned contiguous ranges)
                for off_n, off_q, w in _class_merges(plan):
                    nc.vector.tensor_add(
                        acc4[:, :, off_n : off_n + w],
                        acc4[:, :, off_n : off_n + w],
                        accN4[:, :, off_q : off_q + w],
                    )

            # ---- finish (fp32, small) ----
            icnt = res_pool.tile([P, Q], fp32)
            _raw_scalar_act(nc, icnt[:], cntt[:], AF.Reciprocal)
            div = res_pool.tile([P, Q], fp32)
            nc.vector.tensor_mul(div[:], acc[:, 0:Q], icnt[:])
            acc_div2 = res_pool.tile([P, 1], fp32)
            nc.scalar.activation(div[:], div[:], AF.Square, accum_out=acc_div2[:])
            acc_m = [
                res_pool.tile([P, 1], fp32, tag=f"am{i}", name=f"am{i}")
                for i in range(3)
            ]
            for i in range(3):
                r = res_pool.tile([P, Q], fp32, tag="rfin")
                nc.vector.scalar_tensor_tensor(
                    r[:],
                    acc[:, (1 + i) * Q : (2 + i) * Q],
                    1.0 / REYNOLDS,
                    acc[:, (4 + i) * Q : (5 + i) * Q],
                    OP.mult,
                    OP.add,
                )
                nc.vector.tensor_mul(r[:], r[:], icnt[:])
                nc.scalar.activation(r[:], r[:], AF.Square, accum_out=acc_m[i][:])

            outt = res_pool.tile([P, 8], fp32)
            nc.vector.memset(outt[:], 0.0)
            nc.vector.tensor_copy(outt[:, 0:1], acc_vel[:])
            nc.vector.tensor_copy(outt[:, 1:2], acc_pres[:])
            nc.vector.tensor_copy(outt[:, 2:3], acc_div2[:])
            nc.vector.tensor_copy(outt[:, 3:4], acc_m[0][:])
            nc.vector.tensor_copy(outt[:, 4:5], acc_m[1][:])
            nc.vector.tensor_copy(outt[:, 5:6], acc_m[2][:])
            nc.sync.dma_start(out.ap()[:], outt[:])

    return nc


# ---------------------------------------------------------------- entry

_CACHE = {}


def _get_nc(key, plan, NN, S, Wq, KN, DQ):
    if key not in _CACHE:
        _CACHE[key] = _build_nc(plan, NN, S, Wq, KN, DQ)
    return _CACHE[key]


LAST_RESULT = None  # BassKernelResults of the most recent run (for profiling)


def kernel(pred, target, edge_index, pos, _trace_dir=None):
    global LAST_RESULT
    pred = np.asarray(pred)
    target = np.asarray(target)
    pos = np.asarray(pos).astype(np.float32)
    row = np.asarray(edge_index[0]).astype(np.int64)
    col = np.asarray(edge_index[1]).astype(np.int64)
    n = pred.shape[0]

    plan, deg, dnear, dfar, offs, col_s2, KN = _build_plan(row, col, pos, n)
    nodedata = np.concatenate(
        [pos, pred.astype(np.float32)], axis=1
    )
    cores, NN, S, Wq = _build_streams(
        plan, deg, dnear, dfar, offs, col_s2, KN, nodedata
    )
    Qn = NN // P
    WN = max(Wq, 1) * KN

    # data-loss slices: split all n nodes across cores, pad to mult of 128
    per = -(-n // N_CORES)
    DQ = -(-per // P)
    predf = pred.astype(np.float32)
    targf = target.astype(np.float32)

    in_maps = []
    for c in range(N_CORES):
        lo, hi = c * per, min((c + 1) * per, n)
        ps = np.zeros((P * DQ, 4), np.float32)
        ts = np.zeros((P * DQ, 4), np.float32)
        ps[: hi - lo] = predf[lo:hi]
        ts[: hi - lo] = targf[lo:hi]
        # [P, 4*DQ] with plane-major columns: plane i at cols [i*DQ, (i+1)*DQ)
        dlp = ps.reshape(P, DQ, 4).transpose(0, 2, 1).reshape(P, 4 * DQ)
        dlt = ts.reshape(P, DQ, 4).transpose(0, 2, 1).reshape(P, 4 * DQ)
        cc = cores[c]
        m = dict(
            cnt=np.ascontiguousarray(cc["cnt"]),
            nodb=np.ascontiguousarray(
                cc["nodb"].transpose(1, 0, 2).reshape(P, 7 * Qn)
            ),
            nearA=np.ascontiguousarray(
                cc["nearA"].transpose(1, 0, 2).reshape(P, 7 * (Wq * KN))
            ) if Wq > 0 else np.zeros((P, 7 * WN), np.float32),
            nearB=np.ascontiguousarray(
                cc["nearB"].transpose(1, 0, 2).reshape(P, 7 * (Wq * KN))
            ) if Wq > 0 else np.zeros((P, 7 * WN), np.float32),
            dlp=np.ascontiguousarray(dlp),
            dlt=np.ascontiguousarray(dlt),
        )
        m["colT"] = np.ascontiguousarray(
            cc["colp"].reshape(7 * P, S // P)
        )
        in_maps.append(m)

    key = (tuple((K, mm, w) for K, _, mm, w in plan), NN, S, Wq, KN, DQ)
    plan_geom = [(K, None, mm, w) for K, _, mm, w in plan]
    nc = _get_nc(key, plan_geom, NN, S, Wq, KN, DQ)

    if _trace_dir is not None:
        _install_ntff_hook()
        res = run_bass_kernel_spmd(
            nc, in_maps, core_ids=list(range(N_CORES)), trace=True,
            tmpdir=_trace_dir,
        )
    else:
        res = run_bass_kernel_spmd(nc, in_maps, core_ids=list(range(N_CORES)))
    LAST_RESULT = res

    tot = np.zeros(8, np.float64)
    for c in range(N_CORES):
        tot += res.results[c]["out"].astype(np.float64).sum(axis=0)
    s_vel, s_pres, s_div2, am0, am1, am2 = (
        tot[0], tot[1], tot[2], tot[3], tot[4], tot[5]
    )
    loss = (
        s_vel / (3 * n)
        + s_pres / n
        + LAMBDA_CONT * s_div2 / n
        + LAMBDA_MOM * (am0 + am1 + am2) / (3 * n)
    )
    return np.float32(loss)
